# revision 8
# baseline (speedup 1.0000x reference)
# BEiT-style windowed attention (B=64, N=197, C=768, H=12) on 8 Trainium2
# NeuronCores, data-parallel over batch (8 batches per core).
#
# Per-core pipeline (all layouts "channels on partition, tokens free"):
#   phase 1: qkvT = W1 @ xT  (+bias)        -> q,k tiles (128,1576) bf16,
#            v written per-head into (80,1664) tiles with a ones-row at
#            row 64 (becomes the softmax-denominator column after the
#            DMA transpose).
#   phase 2: per (batch, head): S.T = k.T q  with the rel-pos bias
#            PRELOADED into PSUM via an identity matmul, exp on ACT,
#            P@V with lhsT=[v|1] giving O' (65,197) whose row 64 is the
#            softmax denominator; normalize with reciprocal_approx_fast +
#            a row-broadcast DMA + one DVE multiply per head.
#   phase 3: yT = W2 @ OT (proj), DMA out as (768, 1576) f32 per core.
#
# Host side only shards/transposes inputs, gathers rel_table[rel_index],
# and unshards the output.

import numpy as np
import ml_dtypes

BF16 = ml_dtypes.bfloat16

DIM = 768
H = 12
HD = 64
NTOK = 197
B = 64
NCORES = 8
BL = B // NCORES          # batches per core = 8
T = BL * NTOK             # 1576 tokens per core
SCALE = HD ** -0.5
CH = 394                  # free-dim chunk for the dense matmuls (4*394 = 1576)
NCH = T // CH             # 4
KT0, KT1 = 128, NTOK - 128   # key-token tile sizes (128, 69)
G = 4                     # heads per softmax group
VF = 1664                 # vTh tile free size (>= 7*197 + 256 = 1635)

_cache = {}


def _emit(nc):
    import concourse.mybir as mybir
    import concourse.tile as tile
    from concourse.masks import make_identity

    f32 = mybir.dt.float32
    bf16 = mybir.dt.bfloat16
    AF = mybir.ActivationFunctionType

    xT_d = nc.declare_dram_parameter("xT", [DIM, T], bf16, isOutput=False)
    w1_d = nc.declare_dram_parameter("w1", [DIM, 3 * DIM], bf16, isOutput=False)
    qkvb_d = nc.declare_dram_parameter("qkvb", [128, 18], f32, isOutput=False)
    bT_d = nc.declare_dram_parameter("bT", [128, H, 2 * NTOK], bf16, isOutput=False)
    w2_d = nc.declare_dram_parameter("w2", [DIM, DIM], bf16, isOutput=False)
    vinit_d = nc.declare_dram_parameter("vinit", [16, VF], bf16, isOutput=False)
    z64_d = nc.declare_dram_parameter("z64", [64, VF - T], bf16, isOutput=False)
    yT_d = nc.declare_dram_parameter("yT", [DIM, T], f32, isOutput=True)

    with tile.TileContext(nc) as tc:
        with (
            tc.tile_pool(name="const", bufs=1) as cpool,
            tc.tile_pool(name="qk", bufs=1) as qkpool,
            tc.tile_pool(name="vth", bufs=1) as vpool,
            tc.tile_pool(name="ot", bufs=1) as otpool,
        ):
            ident = cpool.tile([128, 128], bf16, tag="ident")
            make_identity(nc, ident[:])
            qkvb = cpool.tile([128, 18], f32, tag="qkvb")
            nc.sync.dma_start(out=qkvb[:], in_=qkvb_d[:])
            bT = cpool.tile([128, H, 2 * NTOK], bf16, tag="bT")
            nc.sync.dma_start(out=bT[:], in_=bT_d[:])
            w2sb = []
            for i in range(6):
                w = cpool.tile([128, DIM], bf16, tag=f"w2_{i}")
                nc.sync.dma_start(out=w[:], in_=w2_d[128 * i:128 * (i + 1), :])
                w2sb.append(w)

            # q,k tiles: rows 0:1536 of qkvT.  v goes to per-head tiles.
            qkT = []
            for i in range(12):
                t_ = qkpool.tile([128, T], bf16, tag=f"qk{i}")
                qkT.append(t_)
            vTh = []
            for h in range(H):
                t_ = vpool.tile([80, VF], bf16, tag=f"vth{h}")
                vTh.append(t_)
                # row 64 <- ones, rows 65:80 <- zeros (denominator machinery)
                nc.sync.dma_start(out=t_[64:80, :], in_=vinit_d[0:16, :])
                # zero the tail cols of the data rows (read by the transposes)
                nc.sync.dma_start(out=t_[0:64, T:VF], in_=z64_d[:])
            OT = []
            for i in range(6):
                t_ = otpool.tile([128, T], bf16, tag=f"ot{i}")
                OT.append(t_)

            # ---------------- phase 1: qkv projection ----------------
            with (
                tc.tile_pool(name="xw", bufs=1) as xw,
                tc.tile_pool(name="p1ps", bufs=3, space="PSUM") as pp1,
            ):
                xTs = []
                for i in range(6):
                    t_ = xw.tile([128, T], bf16, tag=f"x{i}")
                    nc.sync.dma_start(out=t_[:], in_=xT_d[128 * i:128 * (i + 1), :])
                    xTs.append(t_)
                w1s = []
                for i in range(6):
                    t_ = xw.tile([128, 3 * DIM], bf16, tag=f"w1_{i}")
                    nc.sync.dma_start(out=t_[:], in_=w1_d[128 * i:128 * (i + 1), :])
                    w1s.append(t_)

                nev = 0
                for ot in range(18):
                    for c2 in range(NCH // 2):
                        ps = pp1.tile([128, 2, 512], f32, tag="p1")
                        for j in range(2):
                            ch = 2 * c2 + j
                            for kt in range(6):
                                nc.tensor.matmul(
                                    ps[:, j, 0:CH],
                                    w1s[kt][:, 128 * ot:128 * ot + 128],
                                    xTs[kt][:, CH * ch:CH * (ch + 1)],
                                    start=(kt == 0),
                                    stop=(kt == 5),
                                )
                        src = ps[:, :, 0:CH]
                        if ot < 12:
                            dst = qkT[ot][:, 2 * CH * c2:2 * CH * (c2 + 1)].rearrange(
                                "p (a b) -> p a b", a=2
                            )
                            if nev % 2 == 0:
                                nc.scalar.activation(
                                    dst, src, AF.Identity,
                                    bias=qkvb[:, ot:ot + 1],
                                )
                            else:
                                nc.vector.tensor_scalar_add(
                                    dst, src, qkvb[:, ot:ot + 1]
                                )
                            nev += 1
                        else:
                            for half in range(2):
                                hh = 2 * (ot - 12) + half
                                dst = vTh[hh][0:64,
                                              2 * CH * c2:2 * CH * (c2 + 1)].rearrange(
                                    "p (a b) -> p a b", a=2
                                )
                                s2 = ps[64 * half:64 * (half + 1), :, 0:CH]
                                bias_ap = qkvb[64 * half:64 * (half + 1), ot:ot + 1]
                                if nev % 2 == 0:
                                    nc.scalar.activation(
                                        dst, s2, AF.Identity, bias=bias_ap,
                                    )
                                else:
                                    nc.vector.tensor_scalar_add(dst, s2, bias_ap)
                                nev += 1

            # ---------------- phase 2: attention ----------------
            with (
                tc.tile_pool(name="pS", bufs=1, space="PSUM") as pS,
                tc.tile_pool(name="pO", bufs=1, space="PSUM") as pO,
                tc.tile_pool(name="u4", bufs=2) as upool,
                tc.tile_pool(name="vv", bufs=10) as vvpool,
                tc.tile_pool(name="dn", bufs=2) as dnpool,
                tc.tile_pool(name="db", bufs=2) as dbpool,
            ):
                for b in range(BL):
                    t0 = NTOK * b
                    for g in range(H // G):
                        heads = range(G * g, G * (g + 1))
                        psS = pS.tile([128, G, 512], f32, tag="psS")
                        for i, h in enumerate(heads):
                            qt = qkT[h // 2]
                            kt_ = qkT[6 + h // 2]
                            r0 = 64 * (h % 2)
                            q_ap = qt[r0:r0 + 64, t0:t0 + NTOK]
                            # bias preload (identity matmul streams bias rows)
                            nc.tensor.matmul(
                                psS[:, i, 0:2 * NTOK],
                                ident[:],
                                bT[:, h, 0:2 * NTOK],
                                start=True, stop=False, skip_group_check=True,
                            )
                            nc.tensor.matmul(
                                psS[:, i, 0:NTOK],
                                kt_[r0:r0 + 64, t0:t0 + KT0],
                                q_ap,
                                start=False, stop=False, skip_group_check=True,
                            )
                            nc.tensor.matmul(
                                psS[0:KT1, i, NTOK:2 * NTOK],
                                kt_[r0:r0 + 64, t0 + KT0:t0 + NTOK],
                                q_ap,
                                start=False, stop=True, skip_group_check=True,
                            )
                        u4 = upool.tile([128, G, 2 * NTOK], bf16, tag="u4")
                        nc.scalar.activation(
                            u4[:], psS[:, :, 0:2 * NTOK], AF.Exp
                        )
                        psO = pO.tile([128, G, 512], f32, tag="psO")
                        for i, h in enumerate(heads):
                            vv0 = vvpool.tile([128, 80], bf16, tag="vv")
                            vv1 = vvpool.tile([128, 80], bf16, tag="vv")
                            nc.sync.dma_start_transpose(
                                out=vv0[:], in_=vTh[h][0:80, t0:t0 + 128]
                            )
                            nc.sync.dma_start_transpose(
                                out=vv1[:], in_=vTh[h][0:80, t0 + 128:t0 + 256]
                            )
                            nc.tensor.matmul(
                                psO[0:65, i, 0:NTOK],
                                vv0[:, 0:65],
                                u4[:, i, 0:NTOK],
                                start=True, stop=False, skip_group_check=True,
                            )
                            nc.tensor.matmul(
                                psO[0:65, i, 0:NTOK],
                                vv1[0:KT1, 0:65],
                                u4[0:KT1, i, NTOK:2 * NTOK],
                                start=False, stop=True, skip_group_check=True,
                            )
                        dnc = dnpool.tile([1, G, NTOK], f32, tag="dnc")
                        nc.scalar.activation(
                            dnc[:], psO[64:65, :, 0:NTOK], AF.Copy
                        )
                        dnr = dnpool.tile([1, G, NTOK], f32, tag="dnr")
                        nc.vector.reciprocal_approx_fast(
                            out=dnr[:], in_=dnc[:]
                        )
                        dnb = dbpool.tile([64, G, NTOK], f32, tag="dnb")
                        nc.gpsimd.partition_broadcast(dnb[:], dnr[:])
                        for i, h in enumerate(heads):
                            r0 = 64 * (h % 2)
                            nc.vector.tensor_mul(
                                OT[h // 2][r0:r0 + 64, t0:t0 + NTOK],
                                psO[0:64, i, 0:NTOK],
                                dnb[:, i, :],
                            )

            # ---------------- phase 3: output projection ----------------
            with (
                tc.tile_pool(name="p3ps", bufs=2, space="PSUM") as pp3,
                tc.tile_pool(name="yst", bufs=3) as ypool,
            ):
                for co in range(6):
                    for c2 in range(NCH // 2):
                        ps = pp3.tile([128, 2, 512], f32, tag="p3")
                        for j in range(2):
                            ch = 2 * c2 + j
                            for ci in range(6):
                                nc.tensor.matmul(
                                    ps[:, j, 0:CH],
                                    w2sb[ci][:, 128 * co:128 * co + 128],
                                    OT[ci][:, CH * ch:CH * (ch + 1)],
                                    start=(ci == 0),
                                    stop=(ci == 5),
                                )
                        yst = ypool.tile([128, 2, CH], f32, tag="yst")
                        if (co + c2) % 2 == 0:
                            nc.scalar.activation(
                                yst[:], ps[:, :, 0:CH], AF.Copy
                            )
                        else:
                            nc.vector.tensor_copy(yst[:], ps[:, :, 0:CH])
                        nc.sync.dma_start(
                            out=yT_d[128 * co:128 * (co + 1),
                                     2 * CH * c2:2 * CH * (c2 + 1)].rearrange(
                                "p (a b) -> p a b", a=2
                            ),
                            in_=yst[:],
                        )
    return nc


def build_nc():
    if "nc" not in _cache:
        from concourse import bacc
        nc = bacc.Bacc(None, target_bir_lowering=False, debug=False)
        _emit(nc)
        nc.compile()
        _cache["nc"] = nc
    return _cache["nc"]


def host_prep(x, qkv_w, q_bias, v_bias, rel_table, proj_w, proj_b, rel_index):
    """Shard + lay out inputs for the 8 cores. Returns list of in_maps."""
    x = np.asarray(x, np.float32)
    qkv_w = np.asarray(qkv_w, np.float32)
    q_bias = np.asarray(q_bias, np.float32)
    v_bias = np.asarray(v_bias, np.float32)
    rel_table = np.asarray(rel_table, np.float32)
    proj_w = np.asarray(proj_w, np.float32)
    rel_index = np.asarray(rel_index)

    sv = np.ones((3 * DIM, 1), np.float32)
    sv[:DIM] = SCALE
    w1 = np.ascontiguousarray((qkv_w * sv).T).astype(BF16)        # (768, 2304)
    qb = np.concatenate([q_bias * SCALE, np.zeros(DIM, np.float32), v_bias])
    qkvb = np.ascontiguousarray(qb.reshape(18, 128).T).astype(np.float32)

    bias = rel_table[rel_index]                # (197, 197, H), [q, k, h]
    BT = bias.transpose(2, 1, 0)               # (H, k, q)
    bTdev = np.zeros((128, H, 2 * NTOK), np.float32)
    bTdev[:, :, 0:NTOK] = BT.transpose(1, 0, 2)[0:128]
    bTdev[0:KT1, :, NTOK:2 * NTOK] = BT.transpose(1, 0, 2)[128:NTOK]
    bTdev = bTdev.astype(BF16)

    w2 = np.ascontiguousarray(proj_w.T).astype(BF16)              # (768, 768)
    vinit = np.zeros((16, VF), np.float32)
    vinit[0] = 1.0
    vinit = vinit.astype(BF16)
    z64 = np.zeros((64, VF - T), BF16)

    in_maps = []
    for c in range(NCORES):
        xl = x[BL * c:BL * (c + 1)].reshape(T, DIM)
        xTc = np.ascontiguousarray(xl.T).astype(BF16)
        in_maps.append({
            "xT": xTc, "w1": w1, "qkvb": qkvb, "bT": bTdev,
            "w2": w2, "vinit": vinit, "z64": z64,
        })
    return in_maps


def run_device(in_maps, trace=False, tmpdir=None):
    from concourse.bass_utils import run_bass_kernel_spmd
    nc = build_nc()
    res = run_bass_kernel_spmd(
        nc, in_maps, core_ids=list(range(NCORES)), trace=trace, tmpdir=tmpdir
    )
    return res


def kernel(x, qkv_w, q_bias, v_bias, rel_table, proj_w, proj_b, rel_index):
    in_maps = host_prep(x, qkv_w, q_bias, v_bias, rel_table, proj_w, proj_b,
                        rel_index)
    res = run_device(in_maps)
    y = np.empty((B, NTOK, DIM), np.float32)
    for c in range(NCORES):
        yTc = res.results[c]["yT"]
        y[BL * c:BL * (c + 1)] = yTc.T.reshape(BL, NTOK, DIM)
    proj_b = np.asarray(proj_b, np.float32)
    if np.any(proj_b):
        y += proj_b
    return y


# revision 10
# speedup vs baseline: 1.2976x; 1.2976x over previous
# BEiT-style windowed attention (B=64, N=197, C=768, H=12) on 8 Trainium2
# NeuronCores, data-parallel over batch (8 batches per core).
#
# Per-core pipeline:
#   phase 1: q,k computed channels-on-partition (12 tiles (128,1576) bf16);
#            v computed directly in per-(batch, key-tile) natural layout
#            (16 tiles (128, 12, 65) bf16) with a constant ones-column at
#            [:, :, 64] that turns into the softmax denominator row.
#   phase 2: per (batch, 4-head group): rel-pos bias PRELOADED into PSUM
#            via an identity matmul, S.T = k.T q accumulated on top, exp on
#            ACT (no max-subtraction needed: logits are small), P@V with
#            lhsT=[v|1] -> O' (65,197) whose row 64 is the denominator;
#            normalize: ACT row copy -> DVE reciprocal_approx_fast ->
#            gpsimd partition_broadcast -> DVE multiply per head.
#   phase 3: yT = W2 @ OT (proj), DMA out as (768, 1576) f32 per core.
#
# Host side shards/transposes inputs, gathers rel_table[rel_index], scales
# q by SCALE (folded into W1/q_bias), and unshards the output. v_bias and
# proj_b are exact host-side constant adds (softmax rows sum to 1).

import numpy as np
import ml_dtypes

BF16 = ml_dtypes.bfloat16

DIM = 768
H = 12
HD = 64
NTOK = 197
B = 64
NCORES = 8
BL = B // NCORES          # batches per core = 8
T = BL * NTOK             # 1576 tokens per core
SCALE = HD ** -0.5
CH = 394                  # free-dim chunk for the dense matmuls (4*394 = 1576)
NCH = T // CH             # 4
KT0, KT1 = 128, NTOK - 128   # key-token tile sizes (128, 69)
G = 4                     # heads per softmax group
VCH = 384                 # v output-channel chunk (2*384 = 768)

_cache = {}


def _emit(nc):
    import concourse.mybir as mybir
    import concourse.tile as tile
    from concourse.masks import make_identity

    f32 = mybir.dt.float32
    bf16 = mybir.dt.bfloat16
    AF = mybir.ActivationFunctionType

    xT_d = nc.declare_dram_parameter("xT", [DIM, T], bf16, isOutput=False)
    w1_d = nc.declare_dram_parameter("w1", [DIM, 3 * DIM], bf16, isOutput=False)
    qkvb_d = nc.declare_dram_parameter("qkvb", [128, 12], f32, isOutput=False)
    bT_d = nc.declare_dram_parameter("bT", [128, H, 2 * NTOK], bf16, isOutput=False)
    w2_d = nc.declare_dram_parameter("w2", [DIM, DIM], bf16, isOutput=False)
    yT_d = nc.declare_dram_parameter("yT", [DIM, T], f32, isOutput=True)

    with tile.TileContext(nc) as tc:
        with (
            tc.tile_pool(name="const", bufs=1) as cpool,
            tc.tile_pool(name="qk", bufs=1) as qkpool,
            tc.tile_pool(name="vn", bufs=1) as vpool,
            tc.tile_pool(name="ot", bufs=1) as otpool,
        ):
            ident = cpool.tile([128, 128], bf16, tag="ident")
            make_identity(nc, ident[:])
            qkvb = cpool.tile([128, 12], f32, tag="qkvb")
            nc.sync.dma_start(out=qkvb[:], in_=qkvb_d[:])
            bT = cpool.tile([128, H, 2 * NTOK], bf16, tag="bT")
            nc.sync.dma_start(out=bT[:], in_=bT_d[:])
            w2sb = []
            for i in range(6):
                w = cpool.tile([128, DIM], bf16, tag=f"w2_{i}")
                nc.sync.dma_start(out=w[:], in_=w2_d[128 * i:128 * (i + 1), :])
                w2sb.append(w)

            qkT = [qkpool.tile([128, T], bf16, name=f"qk{i}", tag=f"qk{i}")
                   for i in range(12)]
            # v in natural layout per (batch, key-tile): [kt, head, 64+ones]
            vn = [[vpool.tile([128, H, 65], bf16, name=f"vn{b}_{k}",
                             tag=f"vn{b}_{k}")
                   for k in range(2)] for b in range(BL)]
            for b in range(BL):
                for k in range(2):
                    nc.gpsimd.memset(vn[b][k][:, :, 64:65], 1.0)
            OT = [otpool.tile([128, T], bf16, name=f"ot{i}", tag=f"ot{i}")
                  for i in range(6)]

            # ---------------- phase 1: qkv projection ----------------
            with (
                tc.tile_pool(name="xw", bufs=1) as xw,
                tc.tile_pool(name="p1ps", bufs=3, space="PSUM") as pp1,
            ):
                xTs = []
                for i in range(6):
                    t_ = xw.tile([128, T], bf16, tag=f"x{i}")
                    nc.sync.dma_start(out=t_[:], in_=xT_d[128 * i:128 * (i + 1), :])
                    xTs.append(t_)
                w1s = []
                for i in range(6):
                    t_ = xw.tile([128, 3 * DIM], bf16, tag=f"w1_{i}")
                    nc.sync.dma_start(out=t_[:], in_=w1_d[128 * i:128 * (i + 1), :])
                    w1s.append(t_)

                nev = 0
                # q, k: channels-on-partition o-tiles
                for ot in range(12):
                    for c2 in range(NCH // 2):
                        ps = pp1.tile([128, 2, 512], f32, tag="p1")
                        for j in range(2):
                            ch = 2 * c2 + j
                            for kt in range(6):
                                nc.tensor.matmul(
                                    ps[:, j, 0:CH],
                                    w1s[kt][:, 128 * ot:128 * ot + 128],
                                    xTs[kt][:, CH * ch:CH * (ch + 1)],
                                    start=(kt == 0),
                                    stop=(kt == 5),
                                )
                        src = ps[:, :, 0:CH]
                        dst = qkT[ot][:, 2 * CH * c2:2 * CH * (c2 + 1)].rearrange(
                            "p (a b) -> p a b", a=2
                        )
                        if nev % 2 == 0:
                            nc.scalar.activation(
                                dst, src, AF.Identity, bias=qkvb[:, ot:ot + 1]
                            )
                        else:
                            nc.vector.tensor_scalar_add(dst, src, qkvb[:, ot:ot + 1])
                        nev += 1
                # v: natural layout per (batch, key-tile)
                for b in range(BL):
                    t0 = NTOK * b
                    for k in range(2):
                        m = KT0 if k == 0 else KT1
                        ts_ = t0 + 128 * k
                        ps = pp1.tile([128, 2, 512], f32, tag="p1")
                        for c2 in range(2):
                            for kt in range(6):
                                nc.tensor.matmul(
                                    ps[0:m, c2, 0:VCH],
                                    xTs[kt][:, ts_:ts_ + m],
                                    w1s[kt][:, 2 * DIM + VCH * c2:
                                            2 * DIM + VCH * (c2 + 1)],
                                    start=(kt == 0),
                                    stop=(kt == 5),
                                )
                        for c2 in range(2):
                            src = ps[0:m, c2, 0:VCH].rearrange(
                                "p (a b) -> p a b", a=6
                            )
                            dst = vn[b][k][0:m, 6 * c2:6 * (c2 + 1), 0:64]
                            if nev % 2 == 0:
                                nc.scalar.activation(dst, src, AF.Copy)
                            else:
                                nc.vector.tensor_copy(dst, src)
                            nev += 1

            # ---------------- phase 2: attention ----------------
            with (
                tc.tile_pool(name="pS", bufs=1, space="PSUM") as pS,
                tc.tile_pool(name="pO", bufs=1, space="PSUM") as pO,
                tc.tile_pool(name="u4", bufs=2) as upool,
                tc.tile_pool(name="dn", bufs=2) as dnpool,
                tc.tile_pool(name="db", bufs=2) as dbpool,
            ):
                for b in range(BL):
                    t0 = NTOK * b
                    for g in range(H // G):
                        heads = range(G * g, G * (g + 1))
                        psS = pS.tile([128, G, 512], f32, tag="psS")
                        for i, h in enumerate(heads):
                            qt = qkT[h // 2]
                            kt_ = qkT[6 + h // 2]
                            r0 = 64 * (h % 2)
                            q_ap = qt[r0:r0 + 64, t0:t0 + NTOK]
                            nc.tensor.matmul(
                                psS[:, i, 0:2 * NTOK],
                                ident[:],
                                bT[:, h, 0:2 * NTOK],
                                start=True, stop=False, skip_group_check=True,
                            )
                            nc.tensor.matmul(
                                psS[:, i, 0:NTOK],
                                kt_[r0:r0 + 64, t0:t0 + KT0],
                                q_ap,
                                start=False, stop=False, skip_group_check=True,
                            )
                            nc.tensor.matmul(
                                psS[0:KT1, i, NTOK:2 * NTOK],
                                kt_[r0:r0 + 64, t0 + KT0:t0 + NTOK],
                                q_ap,
                                start=False, stop=True, skip_group_check=True,
                            )
                        u4 = upool.tile([128, G, 2 * NTOK], bf16, tag="u4")
                        nc.scalar.activation(u4[:], psS[:, :, 0:2 * NTOK], AF.Exp)
                        psO = pO.tile([128, G, 512], f32, tag="psO")
                        for i, h in enumerate(heads):
                            nc.tensor.matmul(
                                psO[0:65, i, 0:NTOK],
                                vn[b][0][:, h, 0:65],
                                u4[:, i, 0:NTOK],
                                start=True, stop=False, skip_group_check=True,
                            )
                            nc.tensor.matmul(
                                psO[0:65, i, 0:NTOK],
                                vn[b][1][0:KT1, h, 0:65],
                                u4[0:KT1, i, NTOK:2 * NTOK],
                                start=False, stop=True, skip_group_check=True,
                            )
                        dnc = dnpool.tile([1, G, NTOK], f32, tag="dnc")
                        nc.scalar.activation(
                            dnc[:], psO[64:65, :, 0:NTOK], AF.Copy
                        )
                        dnr = dnpool.tile([1, G, NTOK], f32, tag="dnr")
                        nc.vector.reciprocal_approx_fast(out=dnr[:], in_=dnc[:])
                        dnb = dbpool.tile([64, G, NTOK], f32, tag="dnb")
                        nc.gpsimd.partition_broadcast(dnb[:], dnr[:])
                        for i, h in enumerate(heads):
                            r0 = 64 * (h % 2)
                            nc.vector.tensor_mul(
                                OT[h // 2][r0:r0 + 64, t0:t0 + NTOK],
                                psO[0:64, i, 0:NTOK],
                                dnb[:, i, :],
                            )

            # ---------------- phase 3: output projection ----------------
            with (
                tc.tile_pool(name="p3ps", bufs=2, space="PSUM") as pp3,
                tc.tile_pool(name="yst", bufs=3) as ypool,
            ):
                for co in range(6):
                    for c2 in range(NCH // 2):
                        ps = pp3.tile([128, 2, 512], f32, tag="p3")
                        for j in range(2):
                            ch = 2 * c2 + j
                            for ci in range(6):
                                nc.tensor.matmul(
                                    ps[:, j, 0:CH],
                                    w2sb[ci][:, 128 * co:128 * co + 128],
                                    OT[ci][:, CH * ch:CH * (ch + 1)],
                                    start=(ci == 0),
                                    stop=(ci == 5),
                                )
                        yst = ypool.tile([128, 2, CH], f32, tag="yst")
                        if (co + c2) % 2 == 0:
                            nc.scalar.activation(yst[:], ps[:, :, 0:CH], AF.Copy)
                        else:
                            nc.vector.tensor_copy(yst[:], ps[:, :, 0:CH])
                        nc.sync.dma_start(
                            out=yT_d[128 * co:128 * (co + 1),
                                     2 * CH * c2:2 * CH * (c2 + 1)].rearrange(
                                "p (a b) -> p a b", a=2
                            ),
                            in_=yst[:],
                        )
    return nc


def build_nc():
    if "nc" not in _cache:
        from concourse import bacc
        nc = bacc.Bacc(None, target_bir_lowering=False, debug=False)
        _emit(nc)
        nc.compile()
        _cache["nc"] = nc
    return _cache["nc"]


def host_prep(x, qkv_w, q_bias, v_bias, rel_table, proj_w, proj_b, rel_index):
    """Shard + lay out inputs for the 8 cores. Returns list of in_maps."""
    x = np.asarray(x, np.float32)
    qkv_w = np.asarray(qkv_w, np.float32)
    q_bias = np.asarray(q_bias, np.float32)
    rel_table = np.asarray(rel_table, np.float32)
    rel_index = np.asarray(rel_index)

    sv = np.ones((3 * DIM, 1), np.float32)
    sv[:DIM] = SCALE
    w1 = np.ascontiguousarray((qkv_w * sv).T).astype(BF16)        # (768, 2304)
    # per-partition bias for the q,k o-tiles (k bias is zero by construction;
    # v_bias is added host-side: softmax rows sum to 1)
    qb = np.concatenate([q_bias * SCALE, np.zeros(DIM, np.float32)])
    qkvb = np.ascontiguousarray(qb.reshape(12, 128).T).astype(np.float32)

    bias = rel_table[rel_index]                # (197, 197, H), [q, k, h]
    BT = bias.transpose(2, 1, 0)               # (H, k, q)
    bTdev = np.zeros((128, H, 2 * NTOK), np.float32)
    bTdev[:, :, 0:NTOK] = BT.transpose(1, 0, 2)[0:128]
    bTdev[0:KT1, :, NTOK:2 * NTOK] = BT.transpose(1, 0, 2)[128:NTOK]
    bTdev = bTdev.astype(BF16)

    w2 = np.ascontiguousarray(proj_w.T).astype(BF16)              # (768, 768)

    in_maps = []
    for c in range(NCORES):
        xl = x[BL * c:BL * (c + 1)].reshape(T, DIM)
        xTc = np.ascontiguousarray(xl.T).astype(BF16)
        in_maps.append({
            "xT": xTc, "w1": w1, "qkvb": qkvb, "bT": bTdev, "w2": w2,
        })
    return in_maps


def run_device(in_maps, trace=False, tmpdir=None):
    from concourse.bass_utils import run_bass_kernel_spmd
    nc = build_nc()
    res = run_bass_kernel_spmd(
        nc, in_maps, core_ids=list(range(NCORES)), trace=trace, tmpdir=tmpdir
    )
    return res


def kernel(x, qkv_w, q_bias, v_bias, rel_table, proj_w, proj_b, rel_index):
    in_maps = host_prep(x, qkv_w, q_bias, v_bias, rel_table, proj_w, proj_b,
                        rel_index)
    res = run_device(in_maps)
    y = np.empty((B, NTOK, DIM), np.float32)
    for c in range(NCORES):
        yTc = res.results[c]["yT"]
        y[BL * c:BL * (c + 1)] = yTc.T.reshape(BL, NTOK, DIM)
    # exact host-side constant terms: attn rows sum to 1, so v_bias maps to
    # a constant (v_bias @ proj_w.T); proj_b is a plain add.
    v_bias = np.asarray(v_bias, np.float32)
    proj_b = np.asarray(proj_b, np.float32)
    const = proj_b.copy()
    if np.any(v_bias):
        const = const + v_bias @ np.asarray(proj_w, np.float32).T
    if np.any(const):
        y += const
    return y


# revision 14
# speedup vs baseline: 1.3330x; 1.0273x over previous
# BEiT-style windowed attention (B=64, N=197, C=768, H=12) on 8 Trainium2
# NeuronCores, data-parallel over batch (8 batches per core).
#
# Per-core pipeline:
#   phase 1: q,k computed channels-on-partition (12 tiles (128,1576) bf16);
#            v computed directly in per-(batch, key-tile) natural layout
#            (16 tiles (128, 12, 65) bf16) with a constant ones-column at
#            [:, :, 64] that turns into the softmax denominator row.
#   phase 2: per (batch, 4-head group): rel-pos bias PRELOADED into PSUM
#            via an identity matmul, S.T = k.T q accumulated on top, exp on
#            ACT (no max-subtraction needed: logits are small), P@V with
#            lhsT=[v|1] -> O' (65,197) whose row 64 is the denominator;
#            normalize: ACT row copy -> DVE reciprocal_approx_fast ->
#            gpsimd partition_broadcast -> DVE multiply per head.
#   phase 3: yT = W2 @ OT (proj), DMA out as (768, 1576) f32 per core.
#
# Host side shards/transposes inputs, gathers rel_table[rel_index], scales
# q by SCALE (folded into W1/q_bias), and unshards the output. v_bias and
# proj_b are exact host-side constant adds (softmax rows sum to 1).

import numpy as np
import ml_dtypes

BF16 = ml_dtypes.bfloat16

DIM = 768
H = 12
HD = 64
NTOK = 197
B = 64
NCORES = 8
BL = B // NCORES          # batches per core = 8
T = BL * NTOK             # 1576 tokens per core
SCALE = HD ** -0.5
CH = 394                  # free-dim chunk for the dense matmuls (4*394 = 1576)
NCH = T // CH             # 4
KT0, KT1 = 128, NTOK - 128   # key-token tile sizes (128, 69)
G = 4                     # heads per softmax group
VCH = 384                 # v output-channel chunk (2*384 = 768)

_cache = {}


def _emit(nc):
    import concourse.mybir as mybir
    import concourse.tile as tile
    from concourse.masks import make_identity

    f32 = mybir.dt.float32
    bf16 = mybir.dt.bfloat16
    AF = mybir.ActivationFunctionType

    xT_d = nc.declare_dram_parameter("xT", [DIM, T], bf16, isOutput=False)
    w1_d = nc.declare_dram_parameter("w1", [DIM, 3 * DIM], bf16, isOutput=False)
    qkvb_d = nc.declare_dram_parameter("qkvb", [128, 12], f32, isOutput=False)
    bT_d = nc.declare_dram_parameter("bT", [128, H, 2 * NTOK], bf16, isOutput=False)
    w2_d = nc.declare_dram_parameter("w2", [DIM, DIM], bf16, isOutput=False)
    yT_d = nc.declare_dram_parameter("yT", [DIM, T], f32, isOutput=True)

    with tile.TileContext(nc) as tc:
        with (
            tc.tile_pool(name="const", bufs=1) as cpool,
            tc.tile_pool(name="qk", bufs=1) as qkpool,
            tc.tile_pool(name="vn", bufs=1) as vpool,
            tc.tile_pool(name="ot", bufs=1) as otpool,
        ):
            ident = cpool.tile([128, 128], bf16, tag="ident")
            make_identity(nc, ident[:])
            qkvb = cpool.tile([128, 12], f32, tag="qkvb")
            nc.sync.dma_start(out=qkvb[:], in_=qkvb_d[:])
            bT = cpool.tile([128, H, 2 * NTOK], bf16, tag="bT")
            nc.sync.dma_start(out=bT[:], in_=bT_d[:])
            w2sb = []
            for i in range(6):
                w = cpool.tile([128, DIM], bf16, tag=f"w2_{i}")
                nc.sync.dma_start(out=w[:], in_=w2_d[128 * i:128 * (i + 1), :])
                w2sb.append(w)

            qkT = [qkpool.tile([128, T], bf16, name=f"qk{i}", tag=f"qk{i}")
                   for i in range(12)]
            # v in natural layout per (batch, key-tile): [kt, head, 64+ones]
            vn = [[vpool.tile([128, H, 65], bf16, name=f"vn{b}_{k}",
                             tag=f"vn{b}_{k}")
                   for k in range(2)] for b in range(BL)]
            for b in range(BL):
                for k in range(2):
                    nc.gpsimd.memset(vn[b][k][:, :, 64:65], 1.0)
            OT = [otpool.tile([128, T], bf16, name=f"ot{i}", tag=f"ot{i}")
                  for i in range(6)]

            # ---------------- phase 1: qkv projection ----------------
            with (
                tc.tile_pool(name="xw", bufs=1) as xw,
                tc.tile_pool(name="p1ps", bufs=3, space="PSUM") as pp1,
            ):
                xTs = [xw.tile([128, T], bf16, name=f"x{i}", tag=f"x{i}")
                       for i in range(6)]
                w1s = [xw.tile([128, 3 * DIM], bf16, name=f"w1_{i}",
                               tag=f"w1_{i}") for i in range(6)]
                for i in range(6):
                    nc.sync.dma_start(out=w1s[i][:],
                                      in_=w1_d[128 * i:128 * (i + 1), :])
                    nc.scalar.dma_start(out=xTs[i][:],
                                        in_=xT_d[128 * i:128 * (i + 1), :])

                nev = 0
                # q, k: channels-on-partition o-tiles
                for ot in range(12):
                    for c2 in range(NCH // 2):
                        ps = pp1.tile([128, 2, 512], f32, tag="p1")
                        for j in range(2):
                            ch = 2 * c2 + j
                            for kt in range(6):
                                nc.tensor.matmul(
                                    ps[:, j, 0:CH],
                                    w1s[kt][:, 128 * ot:128 * ot + 128],
                                    xTs[kt][:, CH * ch:CH * (ch + 1)],
                                    start=(kt == 0),
                                    stop=(kt == 5),
                                )
                        src = ps[:, :, 0:CH]
                        dst = qkT[ot][:, 2 * CH * c2:2 * CH * (c2 + 1)].rearrange(
                            "p (a b) -> p a b", a=2
                        )
                        if nev % 2 == 0:
                            nc.scalar.activation(
                                dst, src, AF.Identity, bias=qkvb[:, ot:ot + 1]
                            )
                        else:
                            nc.vector.tensor_scalar_add(dst, src, qkvb[:, ot:ot + 1])
                        nev += 1
                # v: natural layout per (batch, key-tile)
                for b in range(BL):
                    t0 = NTOK * b
                    for k in range(2):
                        m = KT0 if k == 0 else KT1
                        ts_ = t0 + 128 * k
                        ps = pp1.tile([128, 2, 512], f32, tag="p1")
                        for c2 in range(2):
                            for kt in range(6):
                                nc.tensor.matmul(
                                    ps[0:m, c2, 0:VCH],
                                    xTs[kt][:, ts_:ts_ + m],
                                    w1s[kt][:, 2 * DIM + VCH * c2:
                                            2 * DIM + VCH * (c2 + 1)],
                                    start=(kt == 0),
                                    stop=(kt == 5),
                                )
                        for c2 in range(2):
                            src = ps[0:m, c2, 0:VCH].rearrange(
                                "p (a b) -> p a b", a=6
                            )
                            dst = vn[b][k][0:m, 6 * c2:6 * (c2 + 1), 0:64]
                            if nev % 2 == 0:
                                nc.scalar.activation(dst, src, AF.Copy)
                            else:
                                nc.vector.tensor_copy(dst, src)
                            nev += 1

            # ------- phase 2: attention, with proj chunks interleaved -------
            def proj_chunk(pp3, ypool, ch):
                for co in range(6):
                    ps = pp3.tile([128, 512], f32, tag="p3", name="p3")
                    for ci in range(6):
                        nc.tensor.matmul(
                            ps[:, 0:CH],
                            w2sb[ci][:, 128 * co:128 * co + 128],
                            OT[ci][:, CH * ch:CH * (ch + 1)],
                            start=(ci == 0),
                            stop=(ci == 5),
                        )
                    yst = ypool.tile([128, CH], f32, tag="yst", name="yst")
                    if (co + ch) % 2 == 0:
                        nc.scalar.activation(yst[:], ps[:, 0:CH], AF.Copy)
                    else:
                        nc.vector.tensor_copy(yst[:], ps[:, 0:CH])
                    nc.sync.dma_start(
                        out=yT_d[128 * co:128 * (co + 1),
                                 CH * ch:CH * (ch + 1)],
                        in_=yst[:],
                    )

            with (
                tc.tile_pool(name="pS", bufs=1, space="PSUM") as pS,
                tc.tile_pool(name="pO", bufs=1, space="PSUM") as pO,
                tc.tile_pool(name="p3ps", bufs=1, space="PSUM") as pp3,
                tc.tile_pool(name="u4", bufs=2) as upool,
                tc.tile_pool(name="dn", bufs=2) as dnpool,
                tc.tile_pool(name="db", bufs=2) as dbpool,
                tc.tile_pool(name="yst", bufs=3) as ypool,
            ):
                for b in range(BL):
                    t0 = NTOK * b
                    for g in range(H // G):
                        heads = range(G * g, G * (g + 1))
                        psS = pS.tile([128, G, 512], f32, tag="psS")
                        for i, h in enumerate(heads):
                            qt = qkT[h // 2]
                            kt_ = qkT[6 + h // 2]
                            r0 = 64 * (h % 2)
                            q_ap = qt[r0:r0 + 64, t0:t0 + NTOK]
                            nc.tensor.matmul(
                                psS[:, i, 0:2 * NTOK],
                                ident[:],
                                bT[:, h, 0:2 * NTOK],
                                start=True, stop=False, skip_group_check=True,
                            )
                            nc.tensor.matmul(
                                psS[:, i, 0:NTOK],
                                kt_[r0:r0 + 64, t0:t0 + KT0],
                                q_ap,
                                start=False, stop=False, skip_group_check=True,
                            )
                            nc.tensor.matmul(
                                psS[0:KT1, i, NTOK:2 * NTOK],
                                kt_[r0:r0 + 64, t0 + KT0:t0 + NTOK],
                                q_ap,
                                start=False, stop=True, skip_group_check=True,
                            )
                        u4 = upool.tile([128, G, 2 * NTOK], bf16, tag="u4")
                        # exp in halves: PV on kt0 can start while kt1 exps
                        nc.scalar.activation(
                            u4[:, :, 0:NTOK], psS[:, :, 0:NTOK], AF.Exp
                        )
                        # O' packed two heads per PSUM bank: head i -> bank
                        # i//2, col offset 197*(i%2)
                        psO = pO.tile([128, 2, 512], f32, tag="psO")
                        for i, h in enumerate(heads):
                            # start=True clears has_written for the WHOLE
                            # bank, so only the first head of the pair may
                            # set it; the second head's first matmul then
                            # overwrites (bit clear) rather than accumulate.
                            nc.tensor.matmul(
                                psO[0:65, i // 2,
                                    NTOK * (i % 2):NTOK * (i % 2) + NTOK],
                                vn[b][0][:, h, 0:65],
                                u4[:, i, 0:NTOK],
                                start=(i % 2 == 0), stop=False,
                                skip_group_check=True,
                            )
                        nc.scalar.activation(
                            u4[:, :, NTOK:2 * NTOK], psS[:, :, NTOK:2 * NTOK],
                            AF.Exp
                        )
                        for i, h in enumerate(heads):
                            nc.tensor.matmul(
                                psO[0:65, i // 2,
                                    NTOK * (i % 2):NTOK * (i % 2) + NTOK],
                                vn[b][1][0:KT1, h, 0:65],
                                u4[0:KT1, i, NTOK:2 * NTOK],
                                start=False, stop=True, skip_group_check=True,
                            )
                        dnc = dnpool.tile([1, 2, 2 * NTOK], f32, tag="dnc")
                        nc.scalar.activation(
                            dnc[:], psO[64:65, :, 0:2 * NTOK], AF.Copy
                        )
                        dnr = dnpool.tile([1, 2, 2 * NTOK], f32, tag="dnr")
                        nc.vector.reciprocal_approx_fast(out=dnr[:], in_=dnc[:])
                        dnb = dbpool.tile([64, 2, 2 * NTOK], f32, tag="dnb")
                        nc.gpsimd.partition_broadcast(dnb[:], dnr[:])
                        for i, h in enumerate(heads):
                            r0 = 64 * (h % 2)
                            nc.vector.tensor_mul(
                                OT[h // 2][r0:r0 + 64, t0:t0 + NTOK],
                                psO[0:64, i // 2,
                                    NTOK * (i % 2):NTOK * (i % 2) + NTOK],
                                dnb[:, i // 2,
                                    NTOK * (i % 2):NTOK * (i % 2) + NTOK],
                            )
                    # after every odd batch, its 394-wide proj chunk is ready
                    if b % 2 == 1:
                        proj_chunk(pp3, ypool, b // 2)
    return nc


def build_nc():
    if "nc" not in _cache:
        from concourse import bacc
        nc = bacc.Bacc(None, target_bir_lowering=False, debug=False)
        _emit(nc)
        nc.compile()
        _cache["nc"] = nc
    return _cache["nc"]


def host_prep(x, qkv_w, q_bias, v_bias, rel_table, proj_w, proj_b, rel_index):
    """Shard + lay out inputs for the 8 cores. Returns list of in_maps."""
    x = np.asarray(x, np.float32)
    qkv_w = np.asarray(qkv_w, np.float32)
    q_bias = np.asarray(q_bias, np.float32)
    rel_table = np.asarray(rel_table, np.float32)
    rel_index = np.asarray(rel_index)

    sv = np.ones((3 * DIM, 1), np.float32)
    sv[:DIM] = SCALE
    w1 = np.ascontiguousarray((qkv_w * sv).T).astype(BF16)        # (768, 2304)
    # per-partition bias for the q,k o-tiles (k bias is zero by construction;
    # v_bias is added host-side: softmax rows sum to 1)
    qb = np.concatenate([q_bias * SCALE, np.zeros(DIM, np.float32)])
    qkvb = np.ascontiguousarray(qb.reshape(12, 128).T).astype(np.float32)

    bias = rel_table[rel_index]                # (197, 197, H), [q, k, h]
    BT = bias.transpose(2, 1, 0)               # (H, k, q)
    bTdev = np.zeros((128, H, 2 * NTOK), np.float32)
    bTdev[:, :, 0:NTOK] = BT.transpose(1, 0, 2)[0:128]
    bTdev[0:KT1, :, NTOK:2 * NTOK] = BT.transpose(1, 0, 2)[128:NTOK]
    bTdev = bTdev.astype(BF16)

    w2 = np.ascontiguousarray(proj_w.T).astype(BF16)              # (768, 768)

    in_maps = []
    for c in range(NCORES):
        xl = x[BL * c:BL * (c + 1)].reshape(T, DIM)
        xTc = np.ascontiguousarray(xl.T).astype(BF16)
        in_maps.append({
            "xT": xTc, "w1": w1, "qkvb": qkvb, "bT": bTdev, "w2": w2,
        })
    return in_maps


def run_device(in_maps, trace=False, tmpdir=None):
    from concourse.bass_utils import run_bass_kernel_spmd
    nc = build_nc()
    res = run_bass_kernel_spmd(
        nc, in_maps, core_ids=list(range(NCORES)), trace=trace, tmpdir=tmpdir
    )
    return res


def kernel(x, qkv_w, q_bias, v_bias, rel_table, proj_w, proj_b, rel_index):
    in_maps = host_prep(x, qkv_w, q_bias, v_bias, rel_table, proj_w, proj_b,
                        rel_index)
    res = run_device(in_maps)
    y = np.empty((B, NTOK, DIM), np.float32)
    for c in range(NCORES):
        yTc = res.results[c]["yT"]
        y[BL * c:BL * (c + 1)] = yTc.T.reshape(BL, NTOK, DIM)
    # exact host-side constant terms: attn rows sum to 1, so v_bias maps to
    # a constant (v_bias @ proj_w.T); proj_b is a plain add.
    v_bias = np.asarray(v_bias, np.float32)
    proj_b = np.asarray(proj_b, np.float32)
    const = proj_b.copy()
    if np.any(v_bias):
        const = const + v_bias @ np.asarray(proj_w, np.float32).T
    if np.any(const):
        y += const
    return y


# revision 15
# speedup vs baseline: 1.7592x; 1.3197x over previous
# BEiT-style windowed attention (B=64, N=197, C=768, H=12) on 8 Trainium2
# NeuronCores, data-parallel over batch (8 batches per core).
#
# Per-core pipeline:
#   phase 1: q,k computed channels-on-partition (12 tiles (128,1576) bf16);
#            v computed directly in per-(batch, key-tile) natural layout
#            (16 tiles (128, 12, 65) bf16) with a constant ones-column at
#            [:, :, 64] that turns into the softmax denominator row.
#   phase 2: per (batch, 4-head group): rel-pos bias PRELOADED into PSUM
#            via an identity matmul, S.T = k.T q accumulated on top, exp on
#            ACT (no max-subtraction needed: logits are small), P@V with
#            lhsT=[v|1] -> O' (65,197) whose row 64 is the denominator;
#            normalize: ACT row copy -> DVE reciprocal_approx_fast ->
#            gpsimd partition_broadcast -> DVE multiply per head.
#   phase 3: yT = W2 @ OT (proj), DMA out as (768, 1576) f32 per core.
#
# Host side shards/transposes inputs, gathers rel_table[rel_index], scales
# q by SCALE (folded into W1/q_bias), and unshards the output. v_bias and
# proj_b are exact host-side constant adds (softmax rows sum to 1).

import numpy as np
import ml_dtypes

BF16 = ml_dtypes.bfloat16

DIM = 768
H = 12
HD = 64
NTOK = 197
B = 64
NCORES = 8
BL = B // NCORES          # batches per core = 8
T = BL * NTOK             # 1576 tokens per core
SCALE = HD ** -0.5
CH = 394                  # free-dim chunk for the dense matmuls (4*394 = 1576)
NCH = T // CH             # 4
KT0, KT1 = 128, NTOK - 128   # key-token tile sizes (128, 69)
G = 4                     # heads per softmax group
VCH = 384                 # v output-channel chunk (2*384 = 768)

_cache = {}


def _emit(nc):
    import concourse.mybir as mybir
    import concourse.tile as tile
    from concourse.masks import make_identity

    f32 = mybir.dt.float32
    bf16 = mybir.dt.bfloat16
    AF = mybir.ActivationFunctionType

    xT_d = nc.declare_dram_parameter("xT", [DIM, T], bf16, isOutput=False)
    w1_d = nc.declare_dram_parameter("w1", [DIM, 3 * DIM], bf16, isOutput=False)
    qkvb_d = nc.declare_dram_parameter("qkvb", [128, 12], f32, isOutput=False)
    bT_d = nc.declare_dram_parameter("bT", [128, H, 2 * NTOK], bf16, isOutput=False)
    w2_d = nc.declare_dram_parameter("w2", [DIM, DIM], bf16, isOutput=False)
    yT_d = nc.declare_dram_parameter("yT", [DIM, T], f32, isOutput=True)

    with tile.TileContext(nc) as tc:
        with (
            tc.tile_pool(name="const", bufs=1) as cpool,
            tc.tile_pool(name="qk", bufs=1) as qkpool,
            tc.tile_pool(name="vn", bufs=1) as vpool,
            tc.tile_pool(name="ot", bufs=1) as otpool,
        ):
            ident = cpool.tile([128, 128], bf16, tag="ident")
            make_identity(nc, ident[:])
            qkvb = cpool.tile([128, 12], f32, tag="qkvb")
            nc.sync.dma_start(out=qkvb[:], in_=qkvb_d[:])
            bT = cpool.tile([128, H, 2 * NTOK], bf16, tag="bT")
            nc.sync.dma_start(out=bT[:], in_=bT_d[:])
            w2sb = []
            for i in range(6):
                w = cpool.tile([128, DIM], bf16, tag=f"w2_{i}")
                nc.sync.dma_start(out=w[:], in_=w2_d[128 * i:128 * (i + 1), :])
                w2sb.append(w)

            qkT = [qkpool.tile([128, T], bf16, name=f"qk{i}", tag=f"qk{i}")
                   for i in range(12)]
            # v in natural layout per (batch, key-tile): [kt, head, 64+ones]
            vn = [[vpool.tile([128, H, 65], bf16, name=f"vn{b}_{k}",
                             tag=f"vn{b}_{k}")
                   for k in range(2)] for b in range(BL)]
            for b in range(BL):
                for k in range(2):
                    nc.gpsimd.memset(vn[b][k][:, :, 64:65], 1.0)
            OT = [otpool.tile([128, T], bf16, name=f"ot{i}", tag=f"ot{i}")
                  for i in range(6)]

            # ---------------- phase 1: qkv projection ----------------
            with (
                tc.tile_pool(name="xw", bufs=1) as xw,
                tc.tile_pool(name="p1ps", bufs=3, space="PSUM") as pp1,
            ):
                xTs = [xw.tile([128, T], bf16, name=f"x{i}", tag=f"x{i}")
                       for i in range(6)]
                w1s = [xw.tile([128, 3 * DIM], bf16, name=f"w1_{i}",
                               tag=f"w1_{i}") for i in range(6)]
                for i in range(6):
                    nc.sync.dma_start(out=w1s[i][:],
                                      in_=w1_d[128 * i:128 * (i + 1), :])
                    nc.scalar.dma_start(out=xTs[i][:],
                                        in_=xT_d[128 * i:128 * (i + 1), :])

                nev = 0
                # q, k: channels-on-partition o-tiles
                for ot in range(12):
                    for c2 in range(NCH // 2):
                        ps = pp1.tile([128, 2, 512], f32, tag="p1")
                        for j in range(2):
                            ch = 2 * c2 + j
                            for kt in range(6):
                                nc.tensor.matmul(
                                    ps[:, j, 0:CH],
                                    w1s[kt][:, 128 * ot:128 * ot + 128],
                                    xTs[kt][:, CH * ch:CH * (ch + 1)],
                                    start=(kt == 0),
                                    stop=(kt == 5),
                                )
                        src = ps[:, :, 0:CH]
                        dst = qkT[ot][:, 2 * CH * c2:2 * CH * (c2 + 1)].rearrange(
                            "p (a b) -> p a b", a=2
                        )
                        if nev % 2 == 0:
                            nc.scalar.activation(
                                dst, src, AF.Identity, bias=qkvb[:, ot:ot + 1]
                            )
                        else:
                            nc.vector.tensor_scalar_add(dst, src, qkvb[:, ot:ot + 1])
                        nev += 1
                # v: natural layout per (batch, key-tile)
                for b in range(BL):
                    t0 = NTOK * b
                    for k in range(2):
                        m = KT0 if k == 0 else KT1
                        ts_ = t0 + 128 * k
                        ps = pp1.tile([128, 2, 512], f32, tag="p1")
                        for c2 in range(2):
                            for kt in range(6):
                                nc.tensor.matmul(
                                    ps[0:m, c2, 0:VCH],
                                    xTs[kt][:, ts_:ts_ + m],
                                    w1s[kt][:, 2 * DIM + VCH * c2:
                                            2 * DIM + VCH * (c2 + 1)],
                                    start=(kt == 0),
                                    stop=(kt == 5),
                                )
                        for c2 in range(2):
                            src = ps[0:m, c2, 0:VCH].rearrange(
                                "p (a b) -> p a b", a=6
                            )
                            dst = vn[b][k][0:m, 6 * c2:6 * (c2 + 1), 0:64]
                            if nev % 2 == 0:
                                nc.scalar.activation(dst, src, AF.Copy)
                            else:
                                nc.vector.tensor_copy(dst, src)
                            nev += 1

            # ------- phase 2: attention, with proj chunks interleaved -------
            def proj_chunk(pp3, ypool, ch):
                # one 197-wide column chunk (= one batch) of the projection
                for co in range(6):
                    ps = pp3.tile([128, 512], f32, tag="p3", name="p3")
                    for ci in range(6):
                        nc.tensor.matmul(
                            ps[:, 0:NTOK],
                            w2sb[ci][:, 128 * co:128 * co + 128],
                            OT[ci][:, NTOK * ch:NTOK * (ch + 1)],
                            start=(ci == 0),
                            stop=(ci == 5),
                        )
                    yst = ypool.tile([128, NTOK], f32, tag="yst", name="yst")
                    if (co + ch) % 2 == 0:
                        nc.scalar.activation(yst[:], ps[:, 0:NTOK], AF.Copy)
                    else:
                        nc.vector.tensor_copy(yst[:], ps[:, 0:NTOK])
                    nc.sync.dma_start(
                        out=yT_d[128 * co:128 * (co + 1),
                                 NTOK * ch:NTOK * (ch + 1)],
                        in_=yst[:],
                    )

            with (
                tc.tile_pool(name="pS", bufs=2, space="PSUM") as pS,
                tc.tile_pool(name="pO", bufs=3, space="PSUM") as pO,
                tc.tile_pool(name="p3ps", bufs=1, space="PSUM") as pp3,
                tc.tile_pool(name="u2", bufs=3) as upool,
                tc.tile_pool(name="dn", bufs=3) as dnpool,
                tc.tile_pool(name="db", bufs=3) as dbpool,
                tc.tile_pool(name="yst", bufs=3) as ypool,
            ):
                nsg = 0
                for b in range(BL):
                    t0 = NTOK * b
                    for j in range(6):          # head pair (2j, 2j+1)
                        pair = (2 * j, 2 * j + 1)
                        psS = pS.tile([128, 2, 512], f32, tag="psS")
                        for i, h in enumerate(pair):
                            qt = qkT[j]
                            kt_ = qkT[6 + j]
                            r0 = 64 * i
                            q_ap = qt[r0:r0 + 64, t0:t0 + NTOK]
                            nc.tensor.matmul(
                                psS[:, i, 0:2 * NTOK],
                                ident[:],
                                bT[:, h, 0:2 * NTOK],
                                start=True, stop=False, skip_group_check=True,
                            )
                            nc.tensor.matmul(
                                psS[:, i, 0:NTOK],
                                kt_[r0:r0 + 64, t0:t0 + KT0],
                                q_ap,
                                start=False, stop=False, skip_group_check=True,
                            )
                            nc.tensor.matmul(
                                psS[0:KT1, i, NTOK:2 * NTOK],
                                kt_[r0:r0 + 64, t0 + KT0:t0 + NTOK],
                                q_ap,
                                start=False, stop=True, skip_group_check=True,
                            )
                        u2 = upool.tile([128, 2, 2 * NTOK], bf16, tag="u2")
                        nc.scalar.activation(u2[:], psS[:, :, 0:2 * NTOK],
                                             AF.Exp)
                        # O' for the pair packed into ONE psum bank:
                        # head i at cols [197i, 197i+197). start=True clears
                        # has_written for the whole bank -> only first head
                        # sets it; the second head's first matmul overwrites.
                        psO = pO.tile([128, 512], f32, tag="psO")
                        for i, h in enumerate(pair):
                            nc.tensor.matmul(
                                psO[0:65, NTOK * i:NTOK * i + NTOK],
                                vn[b][0][:, h, 0:65],
                                u2[:, i, 0:NTOK],
                                start=(i == 0), stop=False,
                                skip_group_check=True,
                            )
                        for i, h in enumerate(pair):
                            nc.tensor.matmul(
                                psO[0:65, NTOK * i:NTOK * i + NTOK],
                                vn[b][1][0:KT1, h, 0:65],
                                u2[0:KT1, i, NTOK:2 * NTOK],
                                start=False, stop=(i == 1),
                                skip_group_check=True,
                            )
                        dnc = dnpool.tile([1, 2 * NTOK], f32, tag="dnc")
                        if nsg % 2 == 0:
                            nc.scalar.activation(dnc[:], psO[64:65, 0:2 * NTOK],
                                                 AF.Copy)
                        else:
                            nc.vector.tensor_copy(dnc[:], psO[64:65, 0:2 * NTOK])
                        dnr = dnpool.tile([1, 2 * NTOK], f32, tag="dnr")
                        nc.vector.reciprocal_approx_fast(out=dnr[:], in_=dnc[:])
                        dnb = dbpool.tile([64, 2 * NTOK], f32, tag="dnb")
                        nc.gpsimd.partition_broadcast(dnb[:], dnr[:])
                        for i, h in enumerate(pair):
                            r0 = 64 * i
                            nc.vector.tensor_mul(
                                OT[j][r0:r0 + 64, t0:t0 + NTOK],
                                psO[0:64, NTOK * i:NTOK * i + NTOK],
                                dnb[:, NTOK * i:NTOK * i + NTOK],
                            )
                        nsg += 1
                    proj_chunk(pp3, ypool, b)
    return nc


def build_nc():
    if "nc" not in _cache:
        from concourse import bacc
        nc = bacc.Bacc(None, target_bir_lowering=False, debug=False)
        _emit(nc)
        nc.compile()
        _cache["nc"] = nc
    return _cache["nc"]


def host_prep(x, qkv_w, q_bias, v_bias, rel_table, proj_w, proj_b, rel_index):
    """Shard + lay out inputs for the 8 cores. Returns list of in_maps."""
    x = np.asarray(x, np.float32)
    qkv_w = np.asarray(qkv_w, np.float32)
    q_bias = np.asarray(q_bias, np.float32)
    rel_table = np.asarray(rel_table, np.float32)
    rel_index = np.asarray(rel_index)

    sv = np.ones((3 * DIM, 1), np.float32)
    sv[:DIM] = SCALE
    w1 = np.ascontiguousarray((qkv_w * sv).T).astype(BF16)        # (768, 2304)
    # per-partition bias for the q,k o-tiles (k bias is zero by construction;
    # v_bias is added host-side: softmax rows sum to 1)
    qb = np.concatenate([q_bias * SCALE, np.zeros(DIM, np.float32)])
    qkvb = np.ascontiguousarray(qb.reshape(12, 128).T).astype(np.float32)

    bias = rel_table[rel_index]                # (197, 197, H), [q, k, h]
    BT = bias.transpose(2, 1, 0)               # (H, k, q)
    bTdev = np.zeros((128, H, 2 * NTOK), np.float32)
    bTdev[:, :, 0:NTOK] = BT.transpose(1, 0, 2)[0:128]
    bTdev[0:KT1, :, NTOK:2 * NTOK] = BT.transpose(1, 0, 2)[128:NTOK]
    bTdev = bTdev.astype(BF16)

    w2 = np.ascontiguousarray(proj_w.T).astype(BF16)              # (768, 768)

    in_maps = []
    for c in range(NCORES):
        xl = x[BL * c:BL * (c + 1)].reshape(T, DIM)
        xTc = np.ascontiguousarray(xl.T).astype(BF16)
        in_maps.append({
            "xT": xTc, "w1": w1, "qkvb": qkvb, "bT": bTdev, "w2": w2,
        })
    return in_maps


def run_device(in_maps, trace=False, tmpdir=None):
    from concourse.bass_utils import run_bass_kernel_spmd
    nc = build_nc()
    res = run_bass_kernel_spmd(
        nc, in_maps, core_ids=list(range(NCORES)), trace=trace, tmpdir=tmpdir
    )
    return res


def kernel(x, qkv_w, q_bias, v_bias, rel_table, proj_w, proj_b, rel_index):
    in_maps = host_prep(x, qkv_w, q_bias, v_bias, rel_table, proj_w, proj_b,
                        rel_index)
    res = run_device(in_maps)
    y = np.empty((B, NTOK, DIM), np.float32)
    for c in range(NCORES):
        yTc = res.results[c]["yT"]
        y[BL * c:BL * (c + 1)] = yTc.T.reshape(BL, NTOK, DIM)
    # exact host-side constant terms: attn rows sum to 1, so v_bias maps to
    # a constant (v_bias @ proj_w.T); proj_b is a plain add.
    v_bias = np.asarray(v_bias, np.float32)
    proj_b = np.asarray(proj_b, np.float32)
    const = proj_b.copy()
    if np.any(v_bias):
        const = const + v_bias @ np.asarray(proj_w, np.float32).T
    if np.any(const):
        y += const
    return y


# revision 17
# speedup vs baseline: 1.8128x; 1.0305x over previous
# BEiT-style windowed attention (B=64, N=197, C=768, H=12) on 8 Trainium2
# NeuronCores, data-parallel over batch (8 batches per core).
#
# Per-core pipeline:
#   phase 1: q,k computed channels-on-partition (12 tiles (128,1576) bf16);
#            v computed directly in per-(batch, key-tile) natural layout
#            (16 tiles (128, 12, 65) bf16) with a constant ones-column at
#            [:, :, 64] that turns into the softmax denominator row.
#   phase 2: per (batch, 4-head group): rel-pos bias PRELOADED into PSUM
#            via an identity matmul, S.T = k.T q accumulated on top, exp on
#            ACT (no max-subtraction needed: logits are small), P@V with
#            lhsT=[v|1] -> O' (65,197) whose row 64 is the denominator;
#            normalize: ACT row copy -> DVE reciprocal_approx_fast ->
#            gpsimd partition_broadcast -> DVE multiply per head.
#   phase 3: yT = W2 @ OT (proj), DMA out as (768, 1576) f32 per core.
#
# Host side shards/transposes inputs, gathers rel_table[rel_index], scales
# q by SCALE (folded into W1/q_bias), and unshards the output. v_bias and
# proj_b are exact host-side constant adds (softmax rows sum to 1).

import numpy as np
import ml_dtypes

BF16 = ml_dtypes.bfloat16

DIM = 768
H = 12
HD = 64
NTOK = 197
B = 64
NCORES = 8
BL = B // NCORES          # batches per core = 8
T = BL * NTOK             # 1576 tokens per core
SCALE = HD ** -0.5
CH = 394                  # free-dim chunk for the dense matmuls (4*394 = 1576)
NCH = T // CH             # 4
KT0, KT1 = 128, NTOK - 128   # key-token tile sizes (128, 69)
G = 4                     # heads per softmax group
VCH = 384                 # v output-channel chunk (2*384 = 768)

_cache = {}


def _emit(nc):
    import concourse.mybir as mybir
    import concourse.tile as tile
    from concourse.masks import make_identity

    f32 = mybir.dt.float32
    bf16 = mybir.dt.bfloat16
    AF = mybir.ActivationFunctionType

    xT_d = nc.declare_dram_parameter("xT", [DIM, T], bf16, isOutput=False)
    w1_d = nc.declare_dram_parameter("w1", [DIM, 3 * DIM], bf16, isOutput=False)
    qkvb_d = nc.declare_dram_parameter("qkvb", [128, 12], f32, isOutput=False)
    bT_d = nc.declare_dram_parameter("bT", [128, H, 2 * NTOK], bf16, isOutput=False)
    w2_d = nc.declare_dram_parameter("w2", [DIM, DIM], bf16, isOutput=False)
    yT_d = nc.declare_dram_parameter("yT", [DIM, T], f32, isOutput=True)

    with tile.TileContext(nc) as tc:
        with (
            tc.tile_pool(name="const", bufs=1) as cpool,
            tc.tile_pool(name="qk", bufs=1) as qkpool,
            tc.tile_pool(name="vn", bufs=1) as vpool,
            tc.tile_pool(name="ot", bufs=1) as otpool,
        ):
            ident = cpool.tile([128, 128], bf16, tag="ident")
            make_identity(nc, ident[:])
            qkvb = cpool.tile([128, 12], f32, tag="qkvb")
            bT = cpool.tile([128, H, 2 * NTOK], bf16, tag="bT")
            w2sb = [cpool.tile([128, DIM], bf16, name=f"w2_{i}",
                               tag=f"w2_{i}") for i in range(6)]

            qkT = [qkpool.tile([128, T], bf16, name=f"qk{i}", tag=f"qk{i}")
                   for i in range(12)]
            # v in natural layout per (batch, key-tile): [kt, head, 64+ones]
            vn = [[vpool.tile([128, H, 65], bf16, name=f"vn{b}_{k}",
                             tag=f"vn{b}_{k}")
                   for k in range(2)] for b in range(BL)]
            for b in range(BL):
                for k in range(2):
                    nc.gpsimd.memset(vn[b][k][:, :, 64:65], 1.0)
            OT = [otpool.tile([128, T], bf16, name=f"ot{i}", tag=f"ot{i}")
                  for i in range(6)]

            # ---------------- phase 1: qkv projection ----------------
            with (
                tc.tile_pool(name="xw", bufs=1) as xw,
                tc.tile_pool(name="p1ps", bufs=3, space="PSUM") as pp1,
            ):
                xTs = [xw.tile([128, T], bf16, name=f"x{i}", tag=f"x{i}")
                       for i in range(6)]
                w1s = [xw.tile([128, 3 * DIM], bf16, name=f"w1_{i}",
                               tag=f"w1_{i}") for i in range(6)]
                for i in range(6):
                    nc.sync.dma_start(out=w1s[i][:],
                                      in_=w1_d[128 * i:128 * (i + 1), :])
                    nc.scalar.dma_start(out=xTs[i][:],
                                        in_=xT_d[128 * i:128 * (i + 1), :])
                # consts are needed only in phase 2/3: queue them behind
                nc.scalar.dma_start(out=bT[:], in_=bT_d[:])
                nc.sync.dma_start(out=qkvb[:], in_=qkvb_d[:])
                for i in range(6):
                    nc.sync.dma_start(out=w2sb[i][:],
                                      in_=w2_d[128 * i:128 * (i + 1), :])

                nev = 0

                def emit_v(b):
                    nonlocal nev
                    t0 = NTOK * b
                    for k in range(2):
                        m = KT0 if k == 0 else KT1
                        ts_ = t0 + 128 * k
                        ps = pp1.tile([128, 2, 512], f32, tag="p1", name="p1")
                        for c2 in range(2):
                            for kt in range(6):
                                nc.tensor.matmul(
                                    ps[0:m, c2, 0:VCH],
                                    xTs[kt][:, ts_:ts_ + m],
                                    w1s[kt][:, 2 * DIM + VCH * c2:
                                            2 * DIM + VCH * (c2 + 1)],
                                    start=(kt == 0),
                                    stop=(kt == 5),
                                )
                        for c2 in range(2):
                            src_ = ps[0:m, c2, 0:VCH].rearrange(
                                "p (a b) -> p a b", a=6
                            )
                            dst = vn[b][k][0:m, 6 * c2:6 * (c2 + 1), 0:64]
                            if nev % 2 == 0:
                                nc.scalar.activation(dst, src_, AF.Copy)
                            else:
                                nc.vector.tensor_copy(dst, src_)
                            nev += 1

                # q, k: channels-on-partition o-tiles, emitted as (q_j, k_j)
                # pairs in subgroup order, with v batches interleaved
                for jp in range(6):
                    for ot in (jp, 6 + jp):
                        for c2 in range(NCH // 2):
                            ps = pp1.tile([128, 2, 512], f32, tag="p1",
                                          name="p1")
                            for jj in range(2):
                                ch = 2 * c2 + jj
                                for kt in range(6):
                                    nc.tensor.matmul(
                                        ps[:, jj, 0:CH],
                                        w1s[kt][:, 128 * ot:128 * ot + 128],
                                        xTs[kt][:, CH * ch:CH * (ch + 1)],
                                        start=(kt == 0),
                                        stop=(kt == 5),
                                    )
                            src = ps[:, :, 0:CH]
                            dst = qkT[ot][:,
                                          2 * CH * c2:2 * CH * (c2 + 1)].rearrange(
                                "p (a b) -> p a b", a=2
                            )
                            if nev % 2 == 0:
                                nc.scalar.activation(
                                    dst, src, AF.Identity,
                                    bias=qkvb[:, ot:ot + 1]
                                )
                            else:
                                nc.vector.tensor_scalar_add(
                                    dst, src, qkvb[:, ot:ot + 1]
                                )
                            nev += 1
                    # first v batches early so attention on batch 0/1
                    # can begin while q/k tiles for later pairs still build
                    if jp < 4:
                        emit_v(2 * jp)
                        emit_v(2 * jp + 1)

            # ------- phase 2: attention, with proj chunks interleaved -------
            def proj_chunk(pp3, ypool, ch):
                # one 197-wide column chunk (= one batch) of the projection
                for co in range(6):
                    ps = pp3.tile([128, 512], f32, tag="p3", name="p3")
                    for ci in range(6):
                        nc.tensor.matmul(
                            ps[:, 0:NTOK],
                            w2sb[ci][:, 128 * co:128 * co + 128],
                            OT[ci][:, NTOK * ch:NTOK * (ch + 1)],
                            start=(ci == 0),
                            stop=(ci == 5),
                        )
                    yst = ypool.tile([128, NTOK], f32, tag="yst", name="yst")
                    if (co + ch) % 2 == 0:
                        nc.scalar.activation(yst[:], ps[:, 0:NTOK], AF.Copy)
                    else:
                        nc.vector.tensor_copy(yst[:], ps[:, 0:NTOK])
                    nc.sync.dma_start(
                        out=yT_d[128 * co:128 * (co + 1),
                                 NTOK * ch:NTOK * (ch + 1)],
                        in_=yst[:],
                    )

            with (
                tc.tile_pool(name="pS", bufs=2, space="PSUM") as pS,
                tc.tile_pool(name="pO", bufs=3, space="PSUM") as pO,
                tc.tile_pool(name="p3ps", bufs=1, space="PSUM") as pp3,
                tc.tile_pool(name="u2", bufs=3) as upool,
                tc.tile_pool(name="dn", bufs=3) as dnpool,
                tc.tile_pool(name="db", bufs=3) as dbpool,
                tc.tile_pool(name="yst", bufs=3) as ypool,
            ):
                nsg = 0
                for b in range(BL):
                    t0 = NTOK * b
                    for j in range(6):          # head pair (2j, 2j+1)
                        pair = (2 * j, 2 * j + 1)
                        psS = pS.tile([128, 2, 512], f32, tag="psS")
                        for i, h in enumerate(pair):
                            qt = qkT[j]
                            kt_ = qkT[6 + j]
                            r0 = 64 * i
                            q_ap = qt[r0:r0 + 64, t0:t0 + NTOK]
                            nc.tensor.matmul(
                                psS[:, i, 0:2 * NTOK],
                                ident[:],
                                bT[:, h, 0:2 * NTOK],
                                start=True, stop=False, skip_group_check=True,
                            )
                            nc.tensor.matmul(
                                psS[:, i, 0:NTOK],
                                kt_[r0:r0 + 64, t0:t0 + KT0],
                                q_ap,
                                start=False, stop=False, skip_group_check=True,
                            )
                            nc.tensor.matmul(
                                psS[0:KT1, i, NTOK:2 * NTOK],
                                kt_[r0:r0 + 64, t0 + KT0:t0 + NTOK],
                                q_ap,
                                start=False, stop=True, skip_group_check=True,
                            )
                        u2 = upool.tile([128, 2, 2 * NTOK], bf16, tag="u2")
                        nc.scalar.activation(u2[:], psS[:, :, 0:2 * NTOK],
                                             AF.Exp)
                        # O' for the pair packed into ONE psum bank:
                        # head i at cols [197i, 197i+197). start=True clears
                        # has_written for the whole bank -> only first head
                        # sets it; the second head's first matmul overwrites.
                        psO = pO.tile([128, 512], f32, tag="psO")
                        for i, h in enumerate(pair):
                            nc.tensor.matmul(
                                psO[0:65, NTOK * i:NTOK * i + NTOK],
                                vn[b][0][:, h, 0:65],
                                u2[:, i, 0:NTOK],
                                start=(i == 0), stop=False,
                                skip_group_check=True,
                            )
                        for i, h in enumerate(pair):
                            nc.tensor.matmul(
                                psO[0:65, NTOK * i:NTOK * i + NTOK],
                                vn[b][1][0:KT1, h, 0:65],
                                u2[0:KT1, i, NTOK:2 * NTOK],
                                start=False, stop=(i == 1),
                                skip_group_check=True,
                            )
                        dnc = dnpool.tile([1, 2 * NTOK], f32, tag="dnc")
                        if nsg % 2 == 0:
                            nc.scalar.activation(dnc[:], psO[64:65, 0:2 * NTOK],
                                                 AF.Copy)
                        else:
                            nc.vector.tensor_copy(dnc[:], psO[64:65, 0:2 * NTOK])
                        dnr = dnpool.tile([1, 2 * NTOK], f32, tag="dnr")
                        nc.vector.reciprocal_approx_fast(out=dnr[:], in_=dnc[:])
                        dnb = dbpool.tile([64, 2 * NTOK], f32, tag="dnb")
                        nc.gpsimd.partition_broadcast(dnb[:], dnr[:])
                        for i, h in enumerate(pair):
                            r0 = 64 * i
                            nc.vector.tensor_mul(
                                OT[j][r0:r0 + 64, t0:t0 + NTOK],
                                psO[0:64, NTOK * i:NTOK * i + NTOK],
                                dnb[:, NTOK * i:NTOK * i + NTOK],
                            )
                        nsg += 1
                    proj_chunk(pp3, ypool, b)
    return nc


def build_nc():
    if "nc" not in _cache:
        from concourse import bacc
        nc = bacc.Bacc(None, target_bir_lowering=False, debug=False)
        _emit(nc)
        nc.compile()
        _cache["nc"] = nc
    return _cache["nc"]


def host_prep(x, qkv_w, q_bias, v_bias, rel_table, proj_w, proj_b, rel_index):
    """Shard + lay out inputs for the 8 cores. Returns list of in_maps."""
    x = np.asarray(x, np.float32)
    qkv_w = np.asarray(qkv_w, np.float32)
    q_bias = np.asarray(q_bias, np.float32)
    rel_table = np.asarray(rel_table, np.float32)
    rel_index = np.asarray(rel_index)

    sv = np.ones((3 * DIM, 1), np.float32)
    sv[:DIM] = SCALE
    w1 = np.ascontiguousarray((qkv_w * sv).T).astype(BF16)        # (768, 2304)
    # per-partition bias for the q,k o-tiles (k bias is zero by construction;
    # v_bias is added host-side: softmax rows sum to 1)
    qb = np.concatenate([q_bias * SCALE, np.zeros(DIM, np.float32)])
    qkvb = np.ascontiguousarray(qb.reshape(12, 128).T).astype(np.float32)

    bias = rel_table[rel_index]                # (197, 197, H), [q, k, h]
    BT = bias.transpose(2, 1, 0)               # (H, k, q)
    bTdev = np.zeros((128, H, 2 * NTOK), np.float32)
    bTdev[:, :, 0:NTOK] = BT.transpose(1, 0, 2)[0:128]
    bTdev[0:KT1, :, NTOK:2 * NTOK] = BT.transpose(1, 0, 2)[128:NTOK]
    bTdev = bTdev.astype(BF16)

    w2 = np.ascontiguousarray(proj_w.T).astype(BF16)              # (768, 768)

    in_maps = []
    for c in range(NCORES):
        xl = x[BL * c:BL * (c + 1)].reshape(T, DIM)
        xTc = np.ascontiguousarray(xl.T).astype(BF16)
        in_maps.append({
            "xT": xTc, "w1": w1, "qkvb": qkvb, "bT": bTdev, "w2": w2,
        })
    return in_maps


def run_device(in_maps, trace=False, tmpdir=None):
    from concourse.bass_utils import run_bass_kernel_spmd
    nc = build_nc()
    res = run_bass_kernel_spmd(
        nc, in_maps, core_ids=list(range(NCORES)), trace=trace, tmpdir=tmpdir
    )
    return res


def kernel(x, qkv_w, q_bias, v_bias, rel_table, proj_w, proj_b, rel_index):
    in_maps = host_prep(x, qkv_w, q_bias, v_bias, rel_table, proj_w, proj_b,
                        rel_index)
    res = run_device(in_maps)
    y = np.empty((B, NTOK, DIM), np.float32)
    for c in range(NCORES):
        yTc = res.results[c]["yT"]
        y[BL * c:BL * (c + 1)] = yTc.T.reshape(BL, NTOK, DIM)
    # exact host-side constant terms: attn rows sum to 1, so v_bias maps to
    # a constant (v_bias @ proj_w.T); proj_b is a plain add.
    v_bias = np.asarray(v_bias, np.float32)
    proj_b = np.asarray(proj_b, np.float32)
    const = proj_b.copy()
    if np.any(v_bias):
        const = const + v_bias @ np.asarray(proj_w, np.float32).T
    if np.any(const):
        y += const
    return y


# revision 18
# speedup vs baseline: 1.8664x; 1.0296x over previous
# BEiT-style windowed attention (B=64, N=197, C=768, H=12) on 8 Trainium2
# NeuronCores, data-parallel over batch (8 batches per core).
#
# Per-core pipeline:
#   phase 1: q,k computed channels-on-partition (12 tiles (128,1576) bf16);
#            v computed directly in per-(batch, key-tile) natural layout
#            (16 tiles (128, 12, 65) bf16) with a constant ones-column at
#            [:, :, 64] that turns into the softmax denominator row.
#   phase 2: per (batch, 4-head group): rel-pos bias PRELOADED into PSUM
#            via an identity matmul, S.T = k.T q accumulated on top, exp on
#            ACT (no max-subtraction needed: logits are small), P@V with
#            lhsT=[v|1] -> O' (65,197) whose row 64 is the denominator;
#            normalize: ACT row copy -> DVE reciprocal_approx_fast ->
#            gpsimd partition_broadcast -> DVE multiply per head.
#   phase 3: yT = W2 @ OT (proj), DMA out as (768, 1576) f32 per core.
#
# Host side shards/transposes inputs, gathers rel_table[rel_index], scales
# q by SCALE (folded into W1/q_bias), and unshards the output. v_bias and
# proj_b are exact host-side constant adds (softmax rows sum to 1).

import numpy as np
import ml_dtypes

BF16 = ml_dtypes.bfloat16

DIM = 768
H = 12
HD = 64
NTOK = 197
B = 64
NCORES = 8
BL = B // NCORES          # batches per core = 8
T = BL * NTOK             # 1576 tokens per core
SCALE = HD ** -0.5
CH = 394                  # free-dim chunk for the dense matmuls (4*394 = 1576)
NCH = T // CH             # 4
KT0, KT1 = 128, NTOK - 128   # key-token tile sizes (128, 69)
G = 4                     # heads per softmax group
VCH = 384                 # v output-channel chunk (2*384 = 768)

_cache = {}


def _emit(nc):
    import concourse.mybir as mybir
    import concourse.tile as tile
    from concourse.masks import make_identity

    f32 = mybir.dt.float32
    bf16 = mybir.dt.bfloat16
    AF = mybir.ActivationFunctionType

    xT_d = nc.declare_dram_parameter("xT", [DIM, T], bf16, isOutput=False)
    w1_d = nc.declare_dram_parameter("w1", [DIM, 3 * DIM], bf16, isOutput=False)
    qkvb_d = nc.declare_dram_parameter("qkvb", [128, 12], f32, isOutput=False)
    bT_d = nc.declare_dram_parameter("bT", [128, H, 2 * NTOK], bf16, isOutput=False)
    w2_d = nc.declare_dram_parameter("w2", [DIM, DIM], bf16, isOutput=False)
    yT_d = nc.declare_dram_parameter("yT", [DIM, T], f32, isOutput=True)

    with tile.TileContext(nc) as tc:
        with (
            tc.tile_pool(name="const", bufs=1) as cpool,
            tc.tile_pool(name="qk", bufs=1) as qkpool,
            tc.tile_pool(name="vn", bufs=1) as vpool,
            tc.tile_pool(name="ot", bufs=1) as otpool,
        ):
            ident = cpool.tile([128, 128], bf16, tag="ident")
            make_identity(nc, ident[:])
            qkvb = cpool.tile([128, 12], f32, tag="qkvb")
            bT = cpool.tile([128, H, 2 * NTOK], bf16, tag="bT")
            w2sb = [cpool.tile([128, DIM], bf16, name=f"w2_{i}",
                               tag=f"w2_{i}") for i in range(6)]

            qkT = [qkpool.tile([128, T], bf16, name=f"qk{i}", tag=f"qk{i}")
                   for i in range(12)]
            # v in natural layout per (batch, key-tile): [kt, head, 64+ones]
            vn = [[vpool.tile([128, H, 65], bf16, name=f"vn{b}_{k}",
                             tag=f"vn{b}_{k}")
                   for k in range(2)] for b in range(BL)]
            for b in range(BL):
                for k in range(2):
                    nc.gpsimd.memset(vn[b][k][:, :, 64:65], 1.0)
            OT = [otpool.tile([128, T], bf16, name=f"ot{i}", tag=f"ot{i}")
                  for i in range(6)]

            # ---------------- phase 1: qkv projection ----------------
            with (
                tc.tile_pool(name="xw", bufs=1) as xw,
                tc.tile_pool(name="p1ps", bufs=2, space="PSUM") as pp1,
            ):
                xTs = [xw.tile([128, T], bf16, name=f"x{i}", tag=f"x{i}")
                       for i in range(6)]
                w1s = [xw.tile([128, 3 * DIM], bf16, name=f"w1_{i}",
                               tag=f"w1_{i}") for i in range(6)]
                for i in range(6):
                    nc.sync.dma_start(out=w1s[i][:],
                                      in_=w1_d[128 * i:128 * (i + 1), :])
                    nc.scalar.dma_start(out=xTs[i][:],
                                        in_=xT_d[128 * i:128 * (i + 1), :])
                # consts are needed only in phase 2/3: queue them behind
                nc.scalar.dma_start(out=bT[:], in_=bT_d[:])
                nc.sync.dma_start(out=qkvb[:], in_=qkvb_d[:])
                for i in range(6):
                    nc.sync.dma_start(out=w2sb[i][:],
                                      in_=w2_d[128 * i:128 * (i + 1), :])

                nev = 0

                def emit_v(b):
                    nonlocal nev
                    t0 = NTOK * b
                    for k in range(2):
                        m = KT0 if k == 0 else KT1
                        ts_ = t0 + 128 * k
                        ps = pp1.tile([128, 2, 512], f32, tag="p1", name="p1")
                        for c2 in range(2):
                            for kt in range(6):
                                nc.tensor.matmul(
                                    ps[0:m, c2, 0:VCH],
                                    xTs[kt][:, ts_:ts_ + m],
                                    w1s[kt][:, 2 * DIM + VCH * c2:
                                            2 * DIM + VCH * (c2 + 1)],
                                    start=(kt == 0),
                                    stop=(kt == 5),
                                )
                        for c2 in range(2):
                            src_ = ps[0:m, c2, 0:VCH].rearrange(
                                "p (a b) -> p a b", a=6
                            )
                            dst = vn[b][k][0:m, 6 * c2:6 * (c2 + 1), 0:64]
                            if nev % 2 == 0:
                                nc.scalar.activation(dst, src_, AF.Copy)
                            else:
                                nc.vector.tensor_copy(dst, src_)
                            nev += 1

                # q, k: channels-on-partition o-tiles, emitted as (q_j, k_j)
                # pairs in subgroup order, with v batches interleaved
                for jp in range(6):
                    for ot in (jp, 6 + jp):
                        for c2 in range(NCH // 2):
                            ps = pp1.tile([128, 2, 512], f32, tag="p1",
                                          name="p1")
                            for jj in range(2):
                                ch = 2 * c2 + jj
                                for kt in range(6):
                                    nc.tensor.matmul(
                                        ps[:, jj, 0:CH],
                                        w1s[kt][:, 128 * ot:128 * ot + 128],
                                        xTs[kt][:, CH * ch:CH * (ch + 1)],
                                        start=(kt == 0),
                                        stop=(kt == 5),
                                    )
                            src = ps[:, :, 0:CH]
                            dst = qkT[ot][:,
                                          2 * CH * c2:2 * CH * (c2 + 1)].rearrange(
                                "p (a b) -> p a b", a=2
                            )
                            if nev % 2 == 0:
                                nc.scalar.activation(
                                    dst, src, AF.Identity,
                                    bias=qkvb[:, ot:ot + 1]
                                )
                            else:
                                nc.vector.tensor_scalar_add(
                                    dst, src, qkvb[:, ot:ot + 1]
                                )
                            nev += 1
                    # first v batches early so attention on batch 0/1
                    # can begin while q/k tiles for later pairs still build
                    if jp < 4:
                        emit_v(2 * jp)
                        emit_v(2 * jp + 1)

            # ------- phase 2: attention, with proj chunks interleaved -------
            def proj_chunk(pp3, ypool, ch):
                # one 197-wide column chunk (= one batch) of the projection
                for co in range(6):
                    ps = pp3.tile([128, 512], f32, tag="p3", name="p3")
                    for ci in range(6):
                        nc.tensor.matmul(
                            ps[:, 0:NTOK],
                            w2sb[ci][:, 128 * co:128 * co + 128],
                            OT[ci][:, NTOK * ch:NTOK * (ch + 1)],
                            start=(ci == 0),
                            stop=(ci == 5),
                        )
                    yst = ypool.tile([128, NTOK], f32, tag="yst", name="yst")
                    if (co + ch) % 2 == 0:
                        nc.scalar.activation(yst[:], ps[:, 0:NTOK], AF.Copy)
                    else:
                        nc.vector.tensor_copy(yst[:], ps[:, 0:NTOK])
                    nc.sync.dma_start(
                        out=yT_d[128 * co:128 * (co + 1),
                                 NTOK * ch:NTOK * (ch + 1)],
                        in_=yst[:],
                    )

            with (
                tc.tile_pool(name="pS", bufs=2, space="PSUM") as pS,
                tc.tile_pool(name="pO", bufs=3, space="PSUM") as pO,
                tc.tile_pool(name="p3ps", bufs=1, space="PSUM") as pp3,
                tc.tile_pool(name="u2", bufs=3) as upool,
                tc.tile_pool(name="dn", bufs=3) as dnpool,
                tc.tile_pool(name="db", bufs=3) as dbpool,
                tc.tile_pool(name="yst", bufs=3) as ypool,
            ):
                nsg = 0
                for b in range(BL):
                    t0 = NTOK * b
                    for j in range(6):          # head pair (2j, 2j+1)
                        pair = (2 * j, 2 * j + 1)
                        psS = pS.tile([128, 2, 512], f32, tag="psS")
                        for i, h in enumerate(pair):
                            nc.tensor.matmul(
                                psS[:, i, 0:2 * NTOK],
                                ident[:],
                                bT[:, h, 0:2 * NTOK],
                                start=True, stop=False, skip_group_check=True,
                            )
                        for i, h in enumerate(pair):
                            qt = qkT[j]
                            kt_ = qkT[6 + j]
                            r0 = 64 * i
                            q_ap = qt[r0:r0 + 64, t0:t0 + NTOK]
                            nc.tensor.matmul(
                                psS[:, i, 0:NTOK],
                                kt_[r0:r0 + 64, t0:t0 + KT0],
                                q_ap,
                                start=False, stop=False, skip_group_check=True,
                            )
                            nc.tensor.matmul(
                                psS[0:KT1, i, NTOK:2 * NTOK],
                                kt_[r0:r0 + 64, t0 + KT0:t0 + NTOK],
                                q_ap,
                                start=False, stop=True, skip_group_check=True,
                            )
                        u2 = upool.tile([128, 2, 2 * NTOK], bf16, tag="u2")
                        nc.scalar.activation(u2[:], psS[:, :, 0:2 * NTOK],
                                             AF.Exp)
                        # O' for the pair packed into ONE psum bank:
                        # head i at cols [197i, 197i+197). start=True clears
                        # has_written for the whole bank -> only first head
                        # sets it; the second head's first matmul overwrites.
                        psO = pO.tile([128, 512], f32, tag="psO")
                        for i, h in enumerate(pair):
                            nc.tensor.matmul(
                                psO[0:65, NTOK * i:NTOK * i + NTOK],
                                vn[b][0][:, h, 0:65],
                                u2[:, i, 0:NTOK],
                                start=(i == 0), stop=False,
                                skip_group_check=True,
                            )
                        for i, h in enumerate(pair):
                            nc.tensor.matmul(
                                psO[0:65, NTOK * i:NTOK * i + NTOK],
                                vn[b][1][0:KT1, h, 0:65],
                                u2[0:KT1, i, NTOK:2 * NTOK],
                                start=False, stop=(i == 1),
                                skip_group_check=True,
                            )
                        dnc = dnpool.tile([1, 2 * NTOK], f32, tag="dnc")
                        if nsg % 2 == 0:
                            nc.scalar.activation(dnc[:], psO[64:65, 0:2 * NTOK],
                                                 AF.Copy)
                        else:
                            nc.vector.tensor_copy(dnc[:], psO[64:65, 0:2 * NTOK])
                        dnr = dnpool.tile([1, 2 * NTOK], f32, tag="dnr")
                        nc.vector.reciprocal_approx_fast(out=dnr[:], in_=dnc[:])
                        dnb = dbpool.tile([64, 2 * NTOK], f32, tag="dnb")
                        nc.gpsimd.partition_broadcast(dnb[:], dnr[:])
                        for i, h in enumerate(pair):
                            r0 = 64 * i
                            nc.vector.tensor_mul(
                                OT[j][r0:r0 + 64, t0:t0 + NTOK],
                                psO[0:64, NTOK * i:NTOK * i + NTOK],
                                dnb[:, NTOK * i:NTOK * i + NTOK],
                            )
                        nsg += 1
                    proj_chunk(pp3, ypool, b)
    return nc


def build_nc():
    if "nc" not in _cache:
        from concourse import bacc
        nc = bacc.Bacc(None, target_bir_lowering=False, debug=False)
        _emit(nc)
        nc.compile()
        _cache["nc"] = nc
    return _cache["nc"]


def host_prep(x, qkv_w, q_bias, v_bias, rel_table, proj_w, proj_b, rel_index):
    """Shard + lay out inputs for the 8 cores. Returns list of in_maps."""
    x = np.asarray(x, np.float32)
    qkv_w = np.asarray(qkv_w, np.float32)
    q_bias = np.asarray(q_bias, np.float32)
    rel_table = np.asarray(rel_table, np.float32)
    rel_index = np.asarray(rel_index)

    sv = np.ones((3 * DIM, 1), np.float32)
    sv[:DIM] = SCALE
    w1 = np.ascontiguousarray((qkv_w * sv).T).astype(BF16)        # (768, 2304)
    # per-partition bias for the q,k o-tiles (k bias is zero by construction;
    # v_bias is added host-side: softmax rows sum to 1)
    qb = np.concatenate([q_bias * SCALE, np.zeros(DIM, np.float32)])
    qkvb = np.ascontiguousarray(qb.reshape(12, 128).T).astype(np.float32)

    bias = rel_table[rel_index]                # (197, 197, H), [q, k, h]
    BT = bias.transpose(2, 1, 0)               # (H, k, q)
    bTdev = np.zeros((128, H, 2 * NTOK), np.float32)
    bTdev[:, :, 0:NTOK] = BT.transpose(1, 0, 2)[0:128]
    bTdev[0:KT1, :, NTOK:2 * NTOK] = BT.transpose(1, 0, 2)[128:NTOK]
    bTdev = bTdev.astype(BF16)

    w2 = np.ascontiguousarray(proj_w.T).astype(BF16)              # (768, 768)

    in_maps = []
    for c in range(NCORES):
        xl = x[BL * c:BL * (c + 1)].reshape(T, DIM)
        xTc = np.ascontiguousarray(xl.T).astype(BF16)
        in_maps.append({
            "xT": xTc, "w1": w1, "qkvb": qkvb, "bT": bTdev, "w2": w2,
        })
    return in_maps


def run_device(in_maps, trace=False, tmpdir=None):
    from concourse.bass_utils import run_bass_kernel_spmd
    nc = build_nc()
    res = run_bass_kernel_spmd(
        nc, in_maps, core_ids=list(range(NCORES)), trace=trace, tmpdir=tmpdir
    )
    return res


def kernel(x, qkv_w, q_bias, v_bias, rel_table, proj_w, proj_b, rel_index):
    in_maps = host_prep(x, qkv_w, q_bias, v_bias, rel_table, proj_w, proj_b,
                        rel_index)
    res = run_device(in_maps)
    y = np.empty((B, NTOK, DIM), np.float32)
    for c in range(NCORES):
        yTc = res.results[c]["yT"]
        y[BL * c:BL * (c + 1)] = yTc.T.reshape(BL, NTOK, DIM)
    # exact host-side constant terms: attn rows sum to 1, so v_bias maps to
    # a constant (v_bias @ proj_w.T); proj_b is a plain add.
    v_bias = np.asarray(v_bias, np.float32)
    proj_b = np.asarray(proj_b, np.float32)
    const = proj_b.copy()
    if np.any(v_bias):
        const = const + v_bias @ np.asarray(proj_w, np.float32).T
    if np.any(const):
        y += const
    return y


# revision 20
# speedup vs baseline: 1.8960x; 1.0158x over previous
# BEiT-style windowed attention (B=64, N=197, C=768, H=12) on 8 Trainium2
# NeuronCores, data-parallel over batch (8 batches per core).
#
# Per-core pipeline:
#   phase 1: q,k computed channels-on-partition (12 tiles (128,1576) bf16);
#            v computed directly in per-(batch, key-tile) natural layout
#            (16 tiles (128, 12, 65) bf16) with a constant ones-column at
#            [:, :, 64] that turns into the softmax denominator row.
#   phase 2: per (batch, 4-head group): rel-pos bias PRELOADED into PSUM
#            via an identity matmul, S.T = k.T q accumulated on top, exp on
#            ACT (no max-subtraction needed: logits are small), P@V with
#            lhsT=[v|1] -> O' (65,197) whose row 64 is the denominator;
#            normalize: ACT row copy -> DVE reciprocal_approx_fast ->
#            gpsimd partition_broadcast -> DVE multiply per head.
#   phase 3: yT = W2 @ OT (proj), DMA out as (768, 1576) f32 per core.
#
# Host side shards/transposes inputs, gathers rel_table[rel_index], scales
# q by SCALE (folded into W1/q_bias), and unshards the output. v_bias and
# proj_b are exact host-side constant adds (softmax rows sum to 1).

import numpy as np
import ml_dtypes

BF16 = ml_dtypes.bfloat16

DIM = 768
H = 12
HD = 64
NTOK = 197
B = 64
NCORES = 8
BL = B // NCORES          # batches per core = 8
T = BL * NTOK             # 1576 tokens per core
SCALE = HD ** -0.5
CH = 394                  # free-dim chunk for the dense matmuls (4*394 = 1576)
NCH = T // CH             # 4
KT0, KT1 = 128, NTOK - 128   # key-token tile sizes (128, 69)
G = 4                     # heads per softmax group
VCH = 384                 # v output-channel chunk (2*384 = 768)

_cache = {}


def _emit(nc):
    import concourse.mybir as mybir
    import concourse.tile as tile
    from concourse.masks import make_identity

    f32 = mybir.dt.float32
    bf16 = mybir.dt.bfloat16
    AF = mybir.ActivationFunctionType

    xT_d = nc.declare_dram_parameter("xT", [DIM, T], bf16, isOutput=False)
    w1_d = nc.declare_dram_parameter("w1", [DIM, 3 * DIM], bf16, isOutput=False)
    qkvb_d = nc.declare_dram_parameter("qkvb", [128, 12], f32, isOutput=False)
    bT_d = nc.declare_dram_parameter("bT", [128, H, 2 * NTOK], bf16, isOutput=False)
    w2_d = nc.declare_dram_parameter("w2", [DIM, DIM], bf16, isOutput=False)
    yT_d = nc.declare_dram_parameter("yT", [DIM, T], f32, isOutput=True)

    with tile.TileContext(nc) as tc:
        with (
            tc.tile_pool(name="const", bufs=1) as cpool,
            tc.tile_pool(name="qk", bufs=1) as qkpool,
            tc.tile_pool(name="vn", bufs=1) as vpool,
            tc.tile_pool(name="ot", bufs=1) as otpool,
        ):
            ident = cpool.tile([128, 128], bf16, tag="ident")
            make_identity(nc, ident[:])
            qkvb = cpool.tile([128, 12], f32, tag="qkvb")
            bT = cpool.tile([128, H, 2 * NTOK], bf16, tag="bT")
            w2sb = [cpool.tile([128, DIM], bf16, name=f"w2_{i}",
                               tag=f"w2_{i}") for i in range(6)]

            qkT = [qkpool.tile([128, T], bf16, name=f"qk{i}", tag=f"qk{i}")
                   for i in range(12)]
            # v in natural layout per (batch, key-tile): [kt, head, 64+ones]
            vn = [[vpool.tile([128, H, 65], bf16, name=f"vn{b}_{k}",
                             tag=f"vn{b}_{k}")
                   for k in range(2)] for b in range(BL)]
            for b in range(BL):
                for k in range(2):
                    nc.gpsimd.memset(vn[b][k][:, :, 64:65], 1.0)
            OT = [otpool.tile([128, T], bf16, name=f"ot{i}", tag=f"ot{i}")
                  for i in range(6)]

            # ---------------- phase 1: qkv projection ----------------
            with (
                tc.tile_pool(name="xw", bufs=1) as xw,
                tc.tile_pool(name="p1ps", bufs=2, space="PSUM") as pp1,
            ):
                xTs = [xw.tile([128, T], bf16, name=f"x{i}", tag=f"x{i}")
                       for i in range(6)]
                w1s = [xw.tile([128, 3 * DIM], bf16, name=f"w1_{i}",
                               tag=f"w1_{i}") for i in range(6)]
                # inputs split across HWDGE (sync, scalar) and SWDGE
                # (vector, gpsimd) queues; consts queue behind the w1/x tiles
                for i in range(6):
                    half = 3 * DIM // 2
                    nc.sync.dma_start(out=w1s[i][:, 0:half],
                                      in_=w1_d[128 * i:128 * (i + 1), 0:half])
                    nc.scalar.dma_start(
                        out=w1s[i][:, half:3 * DIM],
                        in_=w1_d[128 * i:128 * (i + 1), half:3 * DIM])
                    nc.gpsimd.dma_start(out=xTs[i][:],
                                        in_=xT_d[128 * i:128 * (i + 1), :])
                nc.scalar.dma_start(out=bT[:], in_=bT_d[:])
                nc.sync.dma_start(out=qkvb[:], in_=qkvb_d[:])
                for i in range(6):
                    (nc.sync if i % 2 else nc.scalar).dma_start(
                        out=w2sb[i][:], in_=w2_d[128 * i:128 * (i + 1), :])

                nev = 0

                def emit_v(b):
                    nonlocal nev
                    t0 = NTOK * b
                    for k in range(2):
                        m = KT0 if k == 0 else KT1
                        ts_ = t0 + 128 * k
                        ps = pp1.tile([128, 2, 512], f32, tag="p1", name="p1")
                        for c2 in range(2):
                            for kt in range(6):
                                nc.tensor.matmul(
                                    ps[0:m, c2, 0:VCH],
                                    xTs[kt][:, ts_:ts_ + m],
                                    w1s[kt][:, 2 * DIM + VCH * c2:
                                            2 * DIM + VCH * (c2 + 1)],
                                    start=(kt == 0),
                                    stop=(kt == 5),
                                )
                        for c2 in range(2):
                            src_ = ps[0:m, c2, 0:VCH].rearrange(
                                "p (a b) -> p a b", a=6
                            )
                            dst = vn[b][k][0:m, 6 * c2:6 * (c2 + 1), 0:64]
                            if nev % 2 == 0:
                                nc.scalar.activation(dst, src_, AF.Copy)
                            else:
                                nc.vector.tensor_copy(dst, src_)
                            nev += 1

                # q, k: channels-on-partition o-tiles, emitted as (q_j, k_j)
                # pairs in subgroup order, with v batches interleaved
                for jp in range(6):
                    for ot in (jp, 6 + jp):
                        for c2 in range(NCH // 2):
                            ps = pp1.tile([128, 2, 512], f32, tag="p1",
                                          name="p1")
                            for jj in range(2):
                                ch = 2 * c2 + jj
                                for kt in range(6):
                                    nc.tensor.matmul(
                                        ps[:, jj, 0:CH],
                                        w1s[kt][:, 128 * ot:128 * ot + 128],
                                        xTs[kt][:, CH * ch:CH * (ch + 1)],
                                        start=(kt == 0),
                                        stop=(kt == 5),
                                    )
                            src = ps[:, :, 0:CH]
                            dst = qkT[ot][:,
                                          2 * CH * c2:2 * CH * (c2 + 1)].rearrange(
                                "p (a b) -> p a b", a=2
                            )
                            if nev % 2 == 0:
                                nc.scalar.activation(
                                    dst, src, AF.Identity,
                                    bias=qkvb[:, ot:ot + 1]
                                )
                            else:
                                nc.vector.tensor_scalar_add(
                                    dst, src, qkvb[:, ot:ot + 1]
                                )
                            nev += 1
                    # first v batches early so attention on batch 0/1
                    # can begin while q/k tiles for later pairs still build
                    if jp < 4:
                        emit_v(2 * jp)
                        emit_v(2 * jp + 1)

            # ------- phase 2: attention, with proj chunks interleaved -------
            def proj_chunk(pp3, ypool, ch):
                # one 197-wide column chunk (= one batch) of the projection
                for co in range(6):
                    ps = pp3.tile([128, 512], f32, tag="p3", name="p3")
                    for ci in range(6):
                        nc.tensor.matmul(
                            ps[:, 0:NTOK],
                            w2sb[ci][:, 128 * co:128 * co + 128],
                            OT[ci][:, NTOK * ch:NTOK * (ch + 1)],
                            start=(ci == 0),
                            stop=(ci == 5),
                        )
                    yst = ypool.tile([128, NTOK], f32, tag="yst", name="yst")
                    if (co + ch) % 2 == 0:
                        nc.scalar.activation(yst[:], ps[:, 0:NTOK], AF.Copy)
                    else:
                        nc.vector.tensor_copy(yst[:], ps[:, 0:NTOK])
                    nc.sync.dma_start(
                        out=yT_d[128 * co:128 * (co + 1),
                                 NTOK * ch:NTOK * (ch + 1)],
                        in_=yst[:],
                    )

            with (
                tc.tile_pool(name="pS", bufs=2, space="PSUM") as pS,
                tc.tile_pool(name="pO", bufs=3, space="PSUM") as pO,
                tc.tile_pool(name="p3ps", bufs=1, space="PSUM") as pp3,
                tc.tile_pool(name="u2", bufs=3) as upool,
                tc.tile_pool(name="dn", bufs=3) as dnpool,
                tc.tile_pool(name="db", bufs=3) as dbpool,
                tc.tile_pool(name="yst", bufs=3) as ypool,
            ):
                nsg = 0
                for b in range(BL):
                    t0 = NTOK * b
                    for j in range(6):          # head pair (2j, 2j+1)
                        pair = (2 * j, 2 * j + 1)
                        psS = pS.tile([128, 2, 512], f32, tag="psS")
                        for i, h in enumerate(pair):
                            nc.tensor.matmul(
                                psS[:, i, 0:2 * NTOK],
                                ident[:],
                                bT[:, h, 0:2 * NTOK],
                                start=True, stop=False, skip_group_check=True,
                            )
                        for i, h in enumerate(pair):
                            qt = qkT[j]
                            kt_ = qkT[6 + j]
                            r0 = 64 * i
                            q_ap = qt[r0:r0 + 64, t0:t0 + NTOK]
                            nc.tensor.matmul(
                                psS[:, i, 0:NTOK],
                                kt_[r0:r0 + 64, t0:t0 + KT0],
                                q_ap,
                                start=False, stop=False, skip_group_check=True,
                            )
                            nc.tensor.matmul(
                                psS[0:KT1, i, NTOK:2 * NTOK],
                                kt_[r0:r0 + 64, t0 + KT0:t0 + NTOK],
                                q_ap,
                                start=False, stop=True, skip_group_check=True,
                            )
                        u2 = upool.tile([128, 2, 2 * NTOK], bf16, tag="u2")
                        nc.scalar.activation(u2[:], psS[:, :, 0:2 * NTOK],
                                             AF.Exp)
                        # O' for the pair packed into ONE psum bank:
                        # head i at cols [197i, 197i+197). start=True clears
                        # has_written for the whole bank -> only first head
                        # sets it; the second head's first matmul overwrites.
                        psO = pO.tile([128, 512], f32, tag="psO")
                        for i, h in enumerate(pair):
                            nc.tensor.matmul(
                                psO[0:65, NTOK * i:NTOK * i + NTOK],
                                vn[b][0][:, h, 0:65],
                                u2[:, i, 0:NTOK],
                                start=(i == 0), stop=False,
                                skip_group_check=True,
                            )
                        for i, h in enumerate(pair):
                            nc.tensor.matmul(
                                psO[0:65, NTOK * i:NTOK * i + NTOK],
                                vn[b][1][0:KT1, h, 0:65],
                                u2[0:KT1, i, NTOK:2 * NTOK],
                                start=False, stop=(i == 1),
                                skip_group_check=True,
                            )
                        dnc = dnpool.tile([1, 2 * NTOK], f32, tag="dnc")
                        if nsg % 2 == 0:
                            nc.scalar.activation(dnc[:], psO[64:65, 0:2 * NTOK],
                                                 AF.Copy)
                        else:
                            nc.vector.tensor_copy(dnc[:], psO[64:65, 0:2 * NTOK])
                        dnr = dnpool.tile([1, 2 * NTOK], f32, tag="dnr")
                        nc.vector.reciprocal_approx_fast(out=dnr[:], in_=dnc[:])
                        dnb = dbpool.tile([64, 2 * NTOK], f32, tag="dnb")
                        nc.gpsimd.partition_broadcast(dnb[:], dnr[:])
                        for i, h in enumerate(pair):
                            r0 = 64 * i
                            nc.vector.tensor_mul(
                                OT[j][r0:r0 + 64, t0:t0 + NTOK],
                                psO[0:64, NTOK * i:NTOK * i + NTOK],
                                dnb[:, NTOK * i:NTOK * i + NTOK],
                            )
                        nsg += 1
                    proj_chunk(pp3, ypool, b)
    return nc


def build_nc():
    if "nc" not in _cache:
        from concourse import bacc
        nc = bacc.Bacc(None, target_bir_lowering=False, debug=False)
        _emit(nc)
        nc.compile()
        _cache["nc"] = nc
    return _cache["nc"]


def host_prep(x, qkv_w, q_bias, v_bias, rel_table, proj_w, proj_b, rel_index):
    """Shard + lay out inputs for the 8 cores. Returns list of in_maps."""
    x = np.asarray(x, np.float32)
    qkv_w = np.asarray(qkv_w, np.float32)
    q_bias = np.asarray(q_bias, np.float32)
    rel_table = np.asarray(rel_table, np.float32)
    rel_index = np.asarray(rel_index)

    sv = np.ones((3 * DIM, 1), np.float32)
    sv[:DIM] = SCALE
    w1 = np.ascontiguousarray((qkv_w * sv).T).astype(BF16)        # (768, 2304)
    # per-partition bias for the q,k o-tiles (k bias is zero by construction;
    # v_bias is added host-side: softmax rows sum to 1)
    qb = np.concatenate([q_bias * SCALE, np.zeros(DIM, np.float32)])
    qkvb = np.ascontiguousarray(qb.reshape(12, 128).T).astype(np.float32)

    bias = rel_table[rel_index]                # (197, 197, H), [q, k, h]
    BT = bias.transpose(2, 1, 0)               # (H, k, q)
    bTdev = np.zeros((128, H, 2 * NTOK), np.float32)
    bTdev[:, :, 0:NTOK] = BT.transpose(1, 0, 2)[0:128]
    bTdev[0:KT1, :, NTOK:2 * NTOK] = BT.transpose(1, 0, 2)[128:NTOK]
    bTdev = bTdev.astype(BF16)

    w2 = np.ascontiguousarray(proj_w.T).astype(BF16)              # (768, 768)

    in_maps = []
    for c in range(NCORES):
        xl = x[BL * c:BL * (c + 1)].reshape(T, DIM)
        xTc = np.ascontiguousarray(xl.T).astype(BF16)
        in_maps.append({
            "xT": xTc, "w1": w1, "qkvb": qkvb, "bT": bTdev, "w2": w2,
        })
    return in_maps


def run_device(in_maps, trace=False, tmpdir=None):
    from concourse.bass_utils import run_bass_kernel_spmd
    nc = build_nc()
    res = run_bass_kernel_spmd(
        nc, in_maps, core_ids=list(range(NCORES)), trace=trace, tmpdir=tmpdir
    )
    return res


def kernel(x, qkv_w, q_bias, v_bias, rel_table, proj_w, proj_b, rel_index):
    in_maps = host_prep(x, qkv_w, q_bias, v_bias, rel_table, proj_w, proj_b,
                        rel_index)
    res = run_device(in_maps)
    y = np.empty((B, NTOK, DIM), np.float32)
    for c in range(NCORES):
        yTc = res.results[c]["yT"]
        y[BL * c:BL * (c + 1)] = yTc.T.reshape(BL, NTOK, DIM)
    # exact host-side constant terms: attn rows sum to 1, so v_bias maps to
    # a constant (v_bias @ proj_w.T); proj_b is a plain add.
    v_bias = np.asarray(v_bias, np.float32)
    proj_b = np.asarray(proj_b, np.float32)
    const = proj_b.copy()
    if np.any(v_bias):
        const = const + v_bias @ np.asarray(proj_w, np.float32).T
    if np.any(const):
        y += const
    return y
